# revision 36
# baseline (speedup 1.0000x reference)
"""BrainGNN forward pass on 8 Trainium2 NeuronCores, data-parallel over batch.

v2 — restructured for speed over the v1 baseline (552us):

  PE cuts (the v1 bottleneck at ~69% busy):
  - augment A@A, msg2: fp32r (fp22-truncated 1-pass matmuls, 4x fp32) with
    N=400 single-span rhs; walrus requires the feeding tiles to be declared
    float32r (DMA / ACT producers round on write).  Sandbox-validated:
    end-to-end rel err ~3e-3 with 13-bit input truncation; HW measured
    1.9e-3 (gate 2e-2).
  - conv2: fp16 hi/lo split, 3 cross terms in 2 matmuls per M-chunk
    (err ~2^-21); the s*keep pooling scale is folded into the per-partition
    ACT scale of the PSUM->SBUF copy.
  - pool score rows (p.hT) folded into the msg matmuls as a 33rd lhsT
    column (per-node q-dots via DVE mult+reduce), removing the fp32
    matvecs.
  - transposed readout: hT is PE-transposed to node-on-partition layout;
    s*keep and the -BIG drop mask are per-partition tensor_scalar scalars,
    the mean is a tiny N=1 ones-matmul (plus a constant -BIG*ndrop
    correction), the max a pair of tensor_reduces.  This kills the
    srep/skrep/krepB row replications of v1.
  - rank colsum matmuls (csp) -> PE transpose of the rank4 column flags.
  - 1/cnt1 pre-folded into alTr columns host-side; Q inherits a recip1
    column scaling that is undone in the hT2 normalization (x cnt1*recip2).

  Emission is software-pipelined: front(b) = loads+conv1+msg1 (PE-dense)
  is emitted before back(b-1) = pools/aug/conv2/msg2 (latency-heavy), so
  the scheduler can fill back's cross-engine stalls with front matmuls.
  Conv combines run ACT(psum->sbuf) -> Pool TT -> DVE reduce; per-graph
  input DMA rides the two HWDGE rings (SP + ACT).

  Exactness notes: pool1 keep set is flip-critical (a boundary flip costs
  ~0.1 rel err); its score path (conv1, msg1, q-dot, compares, transposes)
  is exact fp32 throughout.  pool2 flips cost <2e-3 (sandbox-measured), so
  its scores may ride the fp32r msg2.  tensor_tensor_reduce is avoided
  entirely — it kills the exec unit on some NRT/ucode builds (measured
  here); plain mult+reduce pairs replace it.
"""

import math
import numpy as np

NCORES = 8
B = 64
BL = B // NCORES          # graphs per core
R = 400
KC = 8                    # K_COMM rank of the per-node weight factorization
D1 = 32
D2 = 32
D3 = 512
K1 = math.ceil(0.9 * R)   # 360
K2 = math.ceil(0.9 * K1)  # 324
EPS = 1e-5
BIG = 2.0                 # masked-max offset; |h| < 0.5 validated on CPU
SHIFT = 0.0625            # pool2 masked-score offset (v1-validated)

# 400 = 3*128 + 16 partition chunks
CH = [(0, 128), (128, 128), (256, 128), (384, 16)]
# N splits that keep fp32 matmuls at <=256 free size
SP = [(0, 200), (200, 200)]


def build_nc(n_cores=NCORES, reps=1):
    import concourse.bass as bass
    import concourse.mybir as mybir
    from concourse import tile

    F32 = mybir.dt.float32
    F32R = mybir.dt.float32r
    F16 = mybir.dt.float16
    BF16 = mybir.dt.bfloat16
    AX = mybir.AxisListType
    OP = mybir.AluOpType
    AF = mybir.ActivationFunctionType

    nc = bass.Bass()

    xl = nc.dram_tensor("xl", [BL, R, R], F32, kind="ExternalInput")
    al = nc.dram_tensor("al", [BL, R, R], F32R, kind="ExternalInput")
    alT = nc.dram_tensor("alT", [BL, R, R], F32, kind="ExternalInput")
    r1cl = nc.dram_tensor("r1cl", [BL, 128, 4], F32, kind="ExternalInput")
    cnl = nc.dram_tensor("cnl", [BL, R], F32, kind="ExternalInput")
    w1a = nc.dram_tensor("w1a", [R, KC], F32, kind="ExternalInput")
    bc1 = nc.dram_tensor("bc1", [R, D1 * KC], F32, kind="ExternalInput")
    b1d = nc.dram_tensor("b1d", [D1], F32, kind="ExternalInput")
    p1d = nc.dram_tensor("p1d", [D1], F32, kind="ExternalInput")
    pb1d = nc.dram_tensor("pb1d", [1], F32, kind="ExternalInput")
    p1repd = nc.dram_tensor("p1repd", [128, D1], F32, kind="ExternalInput")
    p2repd = nc.dram_tensor("p2repd", [128, D2], F32, kind="ExternalInput")
    p2xd = nc.dram_tensor("p2xd", [128, D2 * KC], F32, kind="ExternalInput")
    w2a = nc.dram_tensor("w2a", [R, KC], F32, kind="ExternalInput")
    bc2hh = nc.dram_tensor("bc2hh", [2 * D1, D2 * KC], F16, kind="ExternalInput")
    bc2lo = nc.dram_tensor("bc2lo", [D1, D2 * KC], F16, kind="ExternalInput")
    b2d = nc.dram_tensor("b2d", [D2], F32, kind="ExternalInput")
    p2d = nc.dram_tensor("p2d", [D2], F32, kind="ExternalInput")
    pb2d = nc.dram_tensor("pb2d", [1], F32, kind="ExternalInput")
    fc1wd = nc.dram_tensor("fc1wd", [4 * D1, D2], F32, kind="ExternalInput")
    fc1bd = nc.dram_tensor("fc1bd", [D2], F32, kind="ExternalInput")
    g1d = nc.dram_tensor("g1d", [D2], F32, kind="ExternalInput")
    be1d = nc.dram_tensor("be1d", [D2], F32, kind="ExternalInput")
    fc2wd = nc.dram_tensor("fc2wd", [D2, D3], F32, kind="ExternalInput")
    fc2bd = nc.dram_tensor("fc2bd", [D3], F32, kind="ExternalInput")
    g2d = nc.dram_tensor("g2d", [D3], F32, kind="ExternalInput")
    be2d = nc.dram_tensor("be2d", [D3], F32, kind="ExternalInput")
    fc3wd = nc.dram_tensor("fc3wd", [D3, 2], F32, kind="ExternalInput")
    fc3bd = nc.dram_tensor("fc3bd", [2], F32, kind="ExternalInput")
    outd = nc.dram_tensor("out", [B, 2], F32, kind="ExternalOutput")

    from contextlib import ExitStack

    with tile.TileContext(nc) as tc, ExitStack() as es:
        cons = es.enter_context(tc.tile_pool(name="cons", bufs=1))
        work = es.enter_context(tc.tile_pool(name="work", bufs=2))
        dram = es.enter_context(tc.tile_pool(name="dram", bufs=1, space="DRAM"))
        pbig = es.enter_context(tc.tile_pool(name="pbig", bufs=2, space="PSUM"))
        pg = es.enter_context(tc.tile_pool(name="pg", bufs=2, space="PSUM"))
        pacc = es.enter_context(tc.tile_pool(name="pacc", bufs=2, space="PSUM"))
        prep = es.enter_context(tc.tile_pool(name="prep", bufs=2, space="PSUM"))

        # ---------------- constants / weights ----------------
        ones128 = cons.tile([128, 128], F32, tag="ones128")
        nc.vector.memset(ones128[:], 1.0)
        ones_bf = cons.tile([128, D1], BF16, tag="ones_bf")
        nc.vector.memset(ones_bf[:], 1.0)
        ones_r = cons.tile([1, 128], F32, tag="ones_r")
        nc.vector.memset(ones_r[:], 1.0)

        a1t, a2t, bc1t = [], [], []
        for c, (o, n) in enumerate(CH):
            t = cons.tile([n, KC], F32, tag=f"a1t{c}")
            nc.sync.dma_start(t[:], w1a[o:o + n, :])
            nc.scalar.activation(t[:], t[:], AF.Relu)
            a1t.append(t)
            t2 = cons.tile([n, KC], F32, tag=f"a2t{c}")
            nc.sync.dma_start(t2[:], w2a[o:o + n, :])
            nc.scalar.activation(t2[:], t2[:], AF.Relu)
            a2t.append(t2)
            tb = cons.tile([n, D1 * KC], F32, tag=f"bc1t{c}")
            nc.sync.dma_start(tb[:], bc1[o:o + n, :])
            bc1t.append(tb)
        # expanded per-partition combine weights [n, (D1,KC)] (stride-0 free
        # broadcasts materialized once so Pool reads plain tiles)
        a1x, a2x = [], []
        for c, (o, n) in enumerate(CH):
            t = cons.tile([n, D1 * KC], F32, tag=f"a1x{c}")
            nc.vector.tensor_copy(t[:].rearrange("p (o c) -> p o c", c=KC),
                                  a1t[c][:].unsqueeze(1).broadcast_to((n, D1, KC)))
            a1x.append(t)
            t2 = cons.tile([n, D1 * KC], F32, tag=f"a2x{c}")
            nc.vector.tensor_copy(t2[:].rearrange("p (o c) -> p o c", c=KC),
                                  a2t[c][:].unsqueeze(1).broadcast_to((n, D1, KC)))
            a2x.append(t2)

        bc2ht = cons.tile([2 * D1, D2 * KC], F16, tag="bc2ht")
        nc.sync.dma_start(bc2ht[:], bc2hh[:, :])
        bc2lt = cons.tile([D1, D2 * KC], F16, tag="bc2lt")
        nc.sync.dma_start(bc2lt[:], bc2lo[:, :])

        def colvec(d, name, nrow):
            t = cons.tile([nrow, 1], F32, tag=name)
            nc.sync.dma_start(t[:], d[:].unsqueeze(1))
            return t

        b1t = colvec(b1d, "b1t", D1)
        pb1t = colvec(pb1d, "pb1t", 1)
        b2t = colvec(b2d, "b2t", D2)
        pb2t = colvec(pb2d, "pb2t", 1)
        fc1bt = colvec(fc1bd, "fc1bt", D2)
        g1t = colvec(g1d, "g1t", D2)
        be1t = colvec(be1d, "be1t", D2)
        fc3bt = colvec(fc3bd, "fc3bt", 2)

        # p replicated across partitions for the q-dot TTRs (host-shipped)
        p1rep = cons.tile([128, D1], F32, tag="p1rep")
        nc.sync.dma_start(p1rep[:], p1repd[:, :])
        p2x = cons.tile([128, D2 * KC], F32, tag="p2x")
        nc.sync.dma_start(p2x[:], p2xd[:, :])
        p2rep = cons.tile([128, D2], F32, tag="p2rep")
        nc.sync.dma_start(p2rep[:], p2repd[:, :])

        fc1wt = cons.tile([4 * D1, D2], F32, tag="fc1wt")
        nc.sync.dma_start(fc1wt[:], fc1wd[:, :])
        fc2wt = cons.tile([D2, D3], F32, tag="fc2wt")
        nc.sync.dma_start(fc2wt[:], fc2wd[:, :])
        fc2b4 = cons.tile([128, 4], F32, tag="fc2b4")
        nc.sync.dma_start(fc2b4[:], fc2bd[:].rearrange("(c p) -> p c", p=128))
        g24 = cons.tile([128, 4], F32, tag="g24")
        nc.sync.dma_start(g24[:], g2d[:].rearrange("(c p) -> p c", p=128))
        be24 = cons.tile([128, 4], F32, tag="be24")
        nc.sync.dma_start(be24[:], be2d[:].rearrange("(c p) -> p c", p=128))
        fc3wt = cons.tile([128, 8], F32, tag="fc3wt")
        nc.sync.dma_start(fc3wt[:].rearrange("p (c o) -> p c o", o=2),
                          fc3wd[:, :].rearrange("(c p) o -> p c o", p=128))

        I128 = cons.tile([128, 128], F32, tag="I128")
        nc.gpsimd.affine_select(I128[:], ones128[:], pattern=[[-1, 128]],
                                compare_op=OP.is_equal, fill=0.0,
                                base=0, channel_multiplier=1)
        notI = cons.tile([128, 128], F32, tag="notI")
        nc.gpsimd.affine_select(notI[:], ones128[:], pattern=[[-1, 128]],
                                compare_op=OP.not_equal, fill=0.0,
                                base=0, channel_multiplier=1)

        # per-engine fences: per-graph ops depend on one late const per
        # engine, not on dozens of const producers (ISA caps sync waits)
        pfence = prep.tile([1, 4], F32, tag="prep")
        fence_pe = nc.tensor.matmul(pfence[:1, 0:1], I128[:, 0:1], I128[:, 0:1])
        dscr = cons.tile([1, 4], F32, tag="dscr")
        fence_dv1 = nc.vector.tensor_copy(dscr[:1, 0:1], notI[0:1, 0:1])
        fence_dv2 = nc.vector.tensor_copy(dscr[:1, 1:2], a2x[3][0:1, 0:1])
        fences = {"pe": fence_pe, "dv1": fence_dv1, "dv2": fence_dv2}
        first_b = {}

        ztile = cons.tile([128, BL], F32, tag="ztile")
        nc.vector.memset(ztile[:], 0.0)
        eps128 = cons.tile([128, 1], F32, tag="eps128")
        nc.vector.memset(eps128[:], EPS)
        nshift128 = cons.tile([128, 1], F32, tag="nshift128")
        nc.vector.memset(nshift128[:], -SHIFT)

        def mm_f32_split(out_ap, lhsT_ap, rhs_ap):
            # keep each fp32 matmul at N<=256 so walrus doesn't auto-fp32r
            nc.tensor.matmul(out_ap[:, 0:200], lhsT_ap, rhs_ap[:, 0:200])
            nc.tensor.matmul(out_ap[:, 200:400], lhsT_ap, rhs_ap[:, 200:400])

        # ---------------- per-graph pipeline (1-graph software pipeline:
        # front(b) = loads+conv1+msg1 (PE-dense), back(b) = pools/aug/conv2/
        # msg2 (latency-heavy); emitting front(b+1) before back(b) keeps PE
        # fed during back's cross-engine handoffs) ----------------

        def col_from_row(row_t, name):
            pcol = prep.tile([128, 4], F32, tag="prep")
            nc.vector.memset(pcol[:, 3:4], 0.0)   # pad rows (s_col reads all)
            for ic, (io, inn) in enumerate(CH):
                nc.tensor.transpose(pcol[:inn, ic:ic + 1],
                                    row_t[:, io:io + inn], I128[:1, :1])
            col = work.tile([128, 4], F32, tag=name)
            nc.scalar.activation(col[:], pcol[:], AF.Identity)
            return col

        def row_from_col(col_t, name):
            prow = prep.tile([1, R], F32, tag="prep")
            for ic, (io, inn) in enumerate(CH):
                nc.tensor.transpose(prow[:1, io:io + inn],
                                    col_t[:inn, ic:ic + 1], I128[:inn, :inn])
            row = work.tile([1, R], F32, tag=name)
            nc.vector.tensor_copy(row[:], prow[:])
            return row

        def rank_keep(row_t, col_t, kthresh, kname, want_row=True):
            """keep_col [128,4] (+ keep_row [1,R]) f32 from score row/col."""
            srep = pbig.tile([128, R], F32, tag="pT")
            nc.tensor.matmul(srep[:, 0:200], ones_r[:], row_t[:, 0:200])
            nc.tensor.matmul(srep[:, 200:400], ones_r[:], row_t[:, 200:400])
            rank4 = work.tile([128, 4], F32, tag=f"{kname}_rk")
            nc.vector.memset(rank4[:, 3:4], 999.0)   # pad rows lose the rank
            for ic, (io, inn) in enumerate(CH):
                cmp = work.tile([128, R], BF16, tag="cmp")
                nc.vector.tensor_scalar(cmp[:inn, :], srep[:inn, :],
                                        col_t[:inn, ic:ic + 1],
                                        0.0, op0=OP.is_gt, op1=OP.add,
                                        accum_out=rank4[:inn, ic:ic + 1])
            keep_col = work.tile([128, 4], F32, tag=f"{kname}_col")
            nc.vector.tensor_scalar(keep_col[:], rank4[:], kthresh, None,
                                    op0=OP.is_lt)
            keep_row = row_from_col(keep_col, f"{kname}_row") if want_row else None
            return keep_col, keep_row

        def readout(b, hT_t, skk_col, bB_col, kdiv, ndrop, zoff):
            # transposed masked readout: hkT[n,d] = hT[d,n]*sk[n] for
            # kept, -BIG for dropped; max via TRs, mean via ones-matmul
            tp = prep.tile([128, 128], F32, tag="prep")
            for ic, (io, inn) in enumerate(CH):
                nc.tensor.transpose(tp[:inn, 32 * ic:32 * ic + D1],
                                    hT_t[:, io:io + inn], I128[:D1, :D1])
            hkT = work.tile([128, 128], F32, tag="hkT")
            # chunk-3 pad rows (nodes 400..511) must lose the max and cancel
            # in the mean correction: fill the whole block, TS rewrites [:16]
            nc.vector.memset(hkT[:, 96:128], -BIG)
            for ic, (io, inn) in enumerate(CH):
                nc.vector.tensor_scalar(hkT[:inn, 32 * ic:32 * ic + D1],
                                        tp[:inn, 32 * ic:32 * ic + D1],
                                        skk_col[:inn, ic:ic + 1],
                                        bB_col[:inn, ic:ic + 1],
                                        op0=OP.mult, op1=OP.add)
            mred = prep.tile([D1, 1], F32, tag="prep")
            for ic, (io, inn) in enumerate(CH):
                nc.tensor.matmul(mred[:], hkT[:inn, 32 * ic:32 * ic + D1],
                                 ones128[:inn, 0:1],
                                 start=(ic == 0), stop=(ic == 3))
            nc.vector.tensor_scalar(ztile[zoff + D1:zoff + 2 * D1, b:b + 1],
                                    mred[:], 1.0 / kdiv, BIG * ndrop / kdiv,
                                    op0=OP.mult, op1=OP.add)
            m1 = work.tile([128, D1], F32, tag="m1")
            nc.vector.tensor_reduce(m1[:],
                                    hkT[:].rearrange("p (c o) -> p o c", c=4),
                                    axis=AX.X, op=OP.max)
            mtr = prep.tile([D1, 128], F32, tag="prep")
            nc.tensor.transpose(mtr[:], m1[:], I128[:, :])
            nc.vector.tensor_reduce(ztile[zoff:zoff + D1, b:b + 1], mtr[:],
                                    axis=AX.X, op=OP.max)

        def front(b):
            # input loads on the two HWDGE rings (SP + ACT); Pool kept free
            xt, at, Ts = [], [], []
            for c, (o, n) in enumerate(CH):
                t = work.tile([n, R], F32, tag=f"xt{c}")
                nc.sync.dma_start(t[:], xl[b, o:o + n, :])
                xt.append(t)
                t = work.tile([n, R], F32R, tag=f"at{c}")
                nc.scalar.dma_start(t[:], al[b, o:o + n, :])
                at.append(t)
                t = work.tile([n, R], F32, tag=f"Ts{c}")
                (nc.sync if c % 2 else nc.scalar).dma_start(t[:], alT[b, o:o + n, :])
                Ts.append(t)
            r1c = work.tile([128, 4], F32, tag="r1c")
            nc.sync.dma_start(r1c[:], r1cl[b])
            c1rep = work.tile([D2, R], F32, tag="c1rep")
            nc.sync.dma_start(c1rep[:],
                              cnl[b, :].unsqueeze(0).broadcast_to((D2, R)))

            # conv1 (fp32): G = h @ [B_c], combine on ACT->Pool->DVE
            ht1e = []
            for mc, (mo, mn) in enumerate(CH):
                gp = pg.tile([mn, D1 * KC], F32, tag="pG")
                for dc, (do, dn) in enumerate(CH):
                    mm = nc.tensor.matmul(gp[:], xt[dc][:, mo:mo + mn], bc1t[dc][:],
                                          start=(dc == 0), stop=(dc == 3))
                    first_b.setdefault("g1", mm)
                gs = work.tile([mn, D1 * KC], F32, tag="gs")
                aa = nc.scalar.activation(gs[:], gp[:], AF.Identity)
                first_b.setdefault("gs", aa)
                t = work.tile([mn, D1 + 1], F32, tag=f"ht1_{mc}")
                prod = work.tile([mn, D1 * KC], F32, tag="prod")
                pp = nc.gpsimd.tensor_tensor(prod[:], gs[:], a1x[mc][:], op=OP.mult)
                first_b.setdefault("prod", pp)
                rr = nc.vector.tensor_reduce(t[:, 0:D1],
                                             prod[:].rearrange("p (o c) -> p o c", c=KC),
                                             axis=AX.X, op=OP.add)
                first_b.setdefault("red", rr)
                qd = work.tile([mn, D1], F32, tag="qd")
                nc.vector.tensor_tensor(qd[:], t[:, 0:D1], p1rep[:mn, :],
                                        op=OP.mult)
                nc.vector.tensor_reduce(t[:, D1:D1 + 1], qd[:], axis=AX.X,
                                        op=OP.add)
                ht1e.append(t)

            # msg1 (fp32): rows 0..31 = hT1 pre-bias, row 32 = score row
            msgp = pacc.tile([D1 + 1, R], F32, tag="pacc")
            for jc, (jo, jn) in enumerate(CH):
                for so, sn in SP:
                    nc.tensor.matmul(msgp[:, so:so + sn], ht1e[jc][:],
                                     Ts[jc][:, so:so + sn],
                                     start=(jc == 0 and so == 0),
                                     stop=(jc == 3 and so == 200))
            return dict(at=at, Ts=Ts, r1c=r1c, c1rep=c1rep, msgp=msgp)

        def back(b, st):
            at, Ts, r1c, c1rep, msgp = (st["at"], st["Ts"], st["r1c"],
                                        st["c1rep"], st["msgp"])
            hT1 = work.tile([D1, R], F32, tag="hT1")
            nc.scalar.activation(hT1[:], msgp[0:D1, :], AF.Identity, bias=b1t[:])
            pre_row = work.tile([1, R], F32, tag="pre_row")
            nc.scalar.activation(pre_row[:], msgp[D1:D1 + 1, :], AF.Identity,
                                 bias=pb1t[:])

            pre_col = col_from_row(pre_row, "pre_col")
            keep_col, keep_row = rank_keep(pre_row, pre_col, K1 - 0.5, "k1")

            s_col = work.tile([128, 4], F32, tag="s_col")
            nc.scalar.activation(s_col[:], pre_col[:], AF.Sigmoid)
            skk_col = work.tile([128, 4], F32, tag="skk_col")
            nc.vector.tensor_tensor(skk_col[:], s_col[:], keep_col[:], op=OP.mult)
            bB_col = work.tile([128, 4], F32, tag="bB_col")
            nc.vector.tensor_scalar(bB_col[:], keep_col[:], BIG, -BIG,
                                    op0=OP.mult, op1=OP.add)
            readout(b, hT1, skk_col, bB_col, K1, R - K1, 0)

            # augment (fp32r): QR = (T+I) S (T+I) * diag(recip1)
            kr1 = work.tile([128, 4], F32, tag="kr1")
            nc.vector.tensor_tensor(kr1[:], keep_col[:], r1c[:], op=OP.mult)
            notk_rowb = work.tile([1, R], BF16, tag="notk_rowb")
            nc.vector.tensor_scalar(notk_rowb[:], keep_row[:], 0.5, None,
                                    op0=OP.is_lt)
            wTr = []
            for jc, (jo, jn) in enumerate(CH):
                wt = work.tile([jn, R], F32R, tag=f"wTr{jc}")
                nc.scalar.activation(wt[:], Ts[jc][:], AF.Identity,
                                     scale=keep_col[:jn, jc:jc + 1])
                wTr.append(wt)
            Qs = []
            cnt2p = pacc.tile([D2, R], F32, tag="pacc")
            for uc, (uo, un) in enumerate(CH):
                qp = pbig.tile([un, R], F32, tag="pT")
                for jc, (jo, jn) in enumerate(CH):
                    nc.tensor.matmul(qp[:], at[jc][:, uo:uo + un], wTr[jc][:],
                                     start=(jc == 0), stop=(jc == 3))
                nc.vector.tensor_tensor(qp[:, uo:uo + un], qp[:, uo:uo + un],
                                        notI[:un, :un], op=OP.mult)
                nc.vector.scalar_tensor_tensor(qp[:, uo:uo + un], I128[:un, :un],
                                               kr1[:un, uc:uc + 1],
                                               qp[:, uo:uo + un],
                                               op0=OP.mult, op1=OP.add)
                ind2 = work.tile([un, R], BF16, tag="ind")
                nc.vector.tensor_scalar(ind2[:], qp[:], 0.0,
                                        keep_col[:un, uc:uc + 1],
                                        op0=OP.is_gt, op1=OP.mult)
                nc.tensor.matmul(cnt2p[:], ones_bf[:un, :D2], ind2[:],
                                 start=(uc == 0), stop=False)
                q = work.tile([un, R], F32R, tag=f"Qs{uc}")
                nc.scalar.activation(q[:], qp[:], AF.Identity)
                Qs.append(q)
            # dropped rows: count += 1 so the reciprocal stays finite
            nc.tensor.matmul(cnt2p[:], ones_bf[:1, :D2], notk_rowb[:],
                             start=False, stop=True)
            rec2 = work.tile([D2, R], F32, tag="rec2")
            nc.vector.reciprocal(rec2[:], cnt2p[:])
            f2 = work.tile([D2, R], F32, tag="f2")
            nc.vector.tensor_tensor(f2[:], rec2[:], c1rep[:], op=OP.mult)

            # conv2 (fp16 3-term): hT1 split hi/lo; s*keep folded into the
            # per-partition scale of the PSUM->SBUF copy
            hk16 = work.tile([2 * D1, R], F16, tag="hk16")
            nc.vector.tensor_copy(hk16[0:D1, :], hT1[:])
            nc.vector.tensor_tensor(hk16[D1:2 * D1, :], hT1[:], hk16[0:D1, :],
                                    op=OP.subtract)
            ht2e = []
            for mc, (mo, mn) in enumerate(CH):
                gp2 = pg.tile([mn, D2 * KC], F32, tag="pG")
                nc.tensor.matmul(gp2[:], hk16[:, mo:mo + mn], bc2ht[:],
                                 start=True, stop=False)
                nc.tensor.matmul(gp2[:], hk16[0:D1, mo:mo + mn], bc2lt[:],
                                 start=False, stop=True)
                gs2 = work.tile([mn, D2 * KC], F32, tag="gs")
                nc.scalar.activation(gs2[:], gp2[:], AF.Identity,
                                     scale=skk_col[:mn, mc:mc + 1])
                prod2 = work.tile([mn, D2 * KC], F32, tag="prod")
                nc.gpsimd.tensor_tensor(prod2[:], gs2[:], a2x[mc][:], op=OP.mult)
                t = work.tile([mn, D2 + 1], F32R, tag=f"ht1_{mc}")
                with nc.allow_low_precision("fp32r feed of the msg2 matmul; "
                                            "sandbox-validated"):
                    nc.vector.tensor_reduce(t[:, 0:D2],
                                            prod2[:].rearrange("p (o c) -> p o c", c=KC),
                                            axis=AX.X, op=OP.add)
                qd2 = work.tile([mn, D2 * KC], F32, tag="qd2")
                qsc = work.tile([mn, 1], F32, tag="qsc")
                nc.vector.tensor_tensor(qd2[:], prod2[:], p2x[:mn, :], op=OP.mult)
                nc.vector.tensor_reduce(qsc[:], qd2[:], axis=AX.X, op=OP.add)
                with nc.allow_low_precision("fp32r score column (pool2 ranks "
                                            "tolerate fp22)"):
                    nc.vector.tensor_copy(t[:, D2:D2 + 1], qsc[:])
                ht2e.append(t)

            # msg2 (fp32r)
            msg2p = pacc.tile([D2 + 1, R], F32, tag="pacc")
            for jc, (jo, jn) in enumerate(CH):
                nc.tensor.matmul(msg2p[:], ht2e[jc][:], Qs[jc][:],
                                 start=(jc == 0), stop=(jc == 3))
            hT2m = work.tile([D2, R], F32, tag="hT2m")
            nc.vector.tensor_tensor(hT2m[:], msg2p[0:D2, :], f2[:], op=OP.mult)
            hT2 = work.tile([D2, R], F32, tag="hT1")
            nc.scalar.activation(hT2[:], hT2m[:], AF.Identity, bias=b2t[:])
            pre2r = work.tile([1, R], F32, tag="pre2r")
            nc.vector.tensor_tensor(pre2r[:], msg2p[D2:D2 + 1, :], f2[0:1, :],
                                    op=OP.mult)
            pre2f = work.tile([1, R], F32, tag="pre2f")
            nc.scalar.activation(pre2f[:], pre2r[:], AF.Identity, bias=pb2t[:])

            # pool2 (masked scores; flips here are benign)
            mp_row = work.tile([1, R], F32, tag="mp_row")
            nc.vector.scalar_tensor_tensor(mp_row[:], pre2f[:], SHIFT,
                                           keep_row[:], op0=OP.add, op1=OP.mult)
            mp_col = col_from_row(mp_row, "mp_col")
            keep2_col, _ = rank_keep(mp_row, mp_col, K2 - 0.5, "k2",
                                     want_row=False)
            s2_col = work.tile([128, 4], F32, tag="s_col")
            nc.scalar.activation(s2_col[:], mp_col[:], AF.Sigmoid,
                                 bias=nshift128[:])
            skk2_col = work.tile([128, 4], F32, tag="skk_col")
            nc.vector.tensor_tensor(skk2_col[:], s2_col[:], keep2_col[:],
                                    op=OP.mult)
            bB2_col = work.tile([128, 4], F32, tag="bB_col")
            nc.vector.tensor_scalar(bB2_col[:], keep2_col[:], BIG, -BIG,
                                    op0=OP.mult, op1=OP.add)
            readout(b, hT2, skk2_col, bB2_col, K2, R - K2, 2 * D1)

        seq = [bb for _ in range(reps) for bb in range(BL)]
        pending = None
        for b in seq:
            st = front(b)
            if pending is not None:
                back(*pending)
            pending = (b, st)
        back(*pending)

        from concourse.tile import add_dep_helper
        for k, tgt in (("g1", "pe"), ("red", "dv1"), ("red", "dv2")):
            if k in first_b:
                add_dep_helper(first_b[k].ins, fences[tgt].ins, sync=False,
                               reason="const fence ordering")

        # ---------------- AllGather + head (redundant on every core) --------
        zloc = dram.tile([128, BL], F32)
        zag = dram.tile([128 * n_cores, BL], F32)
        nc.gpsimd.dma_start(zloc[:], ztile[:])
        if n_cores == 1:
            nc.gpsimd.dma_start(zag[:], zloc[:])
        else:
            import concourse.mybir as _mb
            nc.gpsimd.collective_compute(
                "AllGather",
                _mb.AluOpType.bypass,
                replica_groups=[list(range(n_cores))],
                ins=[zloc[:].opt()],
                outs=[zag[:].opt()],
            )
        ZT = cons.tile([128, B], F32, tag="ZT")
        if n_cores == 1:
            nc.vector.memset(ZT[:], 0.0)
            nc.sync.dma_start(ZT[:, 0:BL], zag[:])
        else:
            nc.sync.dma_start(ZT[:].rearrange("p (c b) -> p c b", b=BL),
                              zag[:].rearrange("(c p) b -> p c b", p=128))

        def bn(y, n, gain, beta):
            mu = cons.tile([n, 1], F32, tag="bn_mu")
            nc.vector.tensor_reduce(mu[:], y[:], axis=AX.X, op=OP.add)
            nc.vector.tensor_scalar(mu[:], mu[:], 1.0 / B, None, op0=OP.mult)
            cen = cons.tile([n, B], F32, tag="bn_cen")
            nc.vector.tensor_scalar(cen[:], y[:], mu[:], None, op0=OP.subtract)
            sq = cons.tile([n, B], F32, tag="bn_sq")
            nc.vector.tensor_tensor(sq[:], cen[:], cen[:], op=OP.mult)
            var = cons.tile([n, 1], F32, tag="bn_var")
            nc.vector.tensor_reduce(var[:], sq[:], axis=AX.X, op=OP.add)
            rstd = cons.tile([n, 1], F32, tag="bn_rstd")
            nc.scalar.activation(rstd[:], var[:], AF.Sqrt, bias=eps128[:n, :],
                                 scale=1.0 / B)
            nc.vector.reciprocal(rstd[:], rstd[:])
            gn = cons.tile([n, 1], F32, tag="bn_gn")
            nc.vector.tensor_tensor(gn[:], rstd[:], gain, op=OP.mult)
            nc.vector.tensor_scalar(y[:], cen[:], gn[:], beta, op0=OP.mult, op1=OP.add)

        y1p = pg.tile([D2, B], F32, tag="pG")
        nc.tensor.matmul(y1p[:], fc1wt[:], ZT[:])
        y1 = cons.tile([D2, B], F32, tag="y1")
        nc.scalar.activation(y1[:], y1p[:], AF.Relu, bias=fc1bt[:])
        bn(y1, D2, g1t[:], be1t[:])

        y3p = pacc.tile([2, B], F32, tag="pacc")
        for mc in range(4):
            y2p = pg.tile([128, B], F32, tag="pG")
            nc.tensor.matmul(y2p[:], fc2wt[:, 128 * mc:128 * (mc + 1)], y1[:])
            y2 = cons.tile([128, B], F32, tag="y2")
            nc.scalar.activation(y2[:], y2p[:], AF.Relu, bias=fc2b4[:, mc:mc + 1])
            bn(y2, 128, g24[:, mc:mc + 1], be24[:, mc:mc + 1])
            nc.tensor.matmul(y3p[:], fc3wt[:, 2 * mc:2 * (mc + 1)], y2[:],
                             start=(mc == 0), stop=(mc == 3))
        y3 = cons.tile([2, B], F32, tag="y3")
        nc.scalar.activation(y3[:], y3p[:], AF.Identity, bias=fc3bt[:])
        nc.sync.dma_start(outd[:, :].rearrange("b o -> o b"), y3[:])

    # Walrus' MM descriptor holds a single sync wait; split multi-waits the
    # same way Bacc.compile does, then populate .instr bytes for extended
    # insts (reciprocal etc).
    import bass_rust as _br
    _br.move_matmul_waits_to_ldweights(nc.m)
    _br.generate_event_semaphores(nc)
    mybir.codegen_inst_isa_subclasses(nc)
    return nc


def make_in_maps(inputs, n_cores=NCORES):
    f32 = np.float32
    f16 = np.float16
    x = np.ascontiguousarray(inputs["x"], dtype=f32)
    adj = np.ascontiguousarray(inputs["adj_w"], dtype=f32)
    p1n = (inputs["p1"] / np.linalg.norm(inputs["p1"])).astype(f32)
    p2n = (inputs["p2"] / np.linalg.norm(inputs["p2"])).astype(f32)
    bc2 = np.ascontiguousarray(
        inputs["W2b"].reshape(KC, D1, D2).transpose(1, 2, 0).reshape(D1, D2 * KC), f32)
    bc2h = bc2.astype(f16)
    bc2l = (bc2 - bc2h.astype(f32)).astype(f16)
    shared = {
        "w1a": np.ascontiguousarray(inputs["W1a"], f32),
        "bc1": np.ascontiguousarray(
            inputs["W1b"].reshape(KC, R, D1).transpose(1, 2, 0).reshape(R, D1 * KC), f32),
        "b1d": np.ascontiguousarray(inputs["b1"], f32),
        "p1d": p1n,
        "pb1d": np.array([np.dot(p1n, inputs["b1"].astype(f32))], f32),
        "p1repd": np.ascontiguousarray(np.tile(p1n, (128, 1))),
        "p2repd": np.ascontiguousarray(np.tile(p2n, (128, 1))),
        "p2xd": np.ascontiguousarray(
            np.tile(np.repeat(p2n, KC), (128, 1)).astype(f32)),
        "w2a": np.ascontiguousarray(inputs["W2a"], f32),
        "bc2hh": np.ascontiguousarray(np.concatenate([bc2h, bc2h], 0)),
        "bc2lo": np.ascontiguousarray(bc2l),
        "b2d": np.ascontiguousarray(inputs["b2"], f32),
        "p2d": p2n,
        "pb2d": np.array([np.dot(p2n, inputs["b2"].astype(f32))], f32),
        "fc1wd": np.ascontiguousarray(inputs["fc1_w"], f32),
        "fc1bd": np.ascontiguousarray(inputs["fc1_b"], f32),
        "g1d": np.ascontiguousarray(inputs["g1"], f32),
        "be1d": np.ascontiguousarray(inputs["be1"], f32),
        "fc2wd": np.ascontiguousarray(inputs["fc2_w"], f32),
        "fc2bd": np.ascontiguousarray(inputs["fc2_b"], f32),
        "g2d": np.ascontiguousarray(inputs["g2"], f32),
        "be2d": np.ascontiguousarray(inputs["be2"], f32),
        "fc3wd": np.ascontiguousarray(inputs["fc3_w"], f32),
        "fc3bd": np.ascontiguousarray(inputs["fc3_b"], f32),
    }
    cnt = 1.0 + np.asarray(inputs["adj_mask"], bool).sum(-1).astype(f32)
    rcl = (np.float32(1.0) / cnt).astype(f32)
    eye = np.eye(R, dtype=f32)
    BLc = B // n_cores
    maps = []
    for c in range(n_cores):
        m = dict(shared)
        sl = slice(c * BLc, (c + 1) * BLc)
        aI = adj[sl] + eye
        # (A+I)^T with columns j scaled by recip1[j]
        aIT = aI.transpose(0, 2, 1) * rcl[sl][:, None, :]
        m["xl"] = np.ascontiguousarray(x[sl])
        m["al"] = np.ascontiguousarray(aI)
        m["alT"] = np.ascontiguousarray(aIT.astype(f32))
        # recip1 in [128, 4] col-chunk layout
        r1p = np.zeros((BLc, 512), f32)
        r1p[:, :R] = rcl[sl]
        m["r1cl"] = np.ascontiguousarray(r1p.reshape(BLc, 4, 128).transpose(0, 2, 1))
        m["cnl"] = np.ascontiguousarray(cnt[sl])
        maps.append(m)
    return maps


_CACHED = {}


def _run_sim(in_maps):
    # Fallback executor: 8-core CoreSim of the same BIR.
    from concourse import bass_interp

    nc = build_nc(NCORES)
    sim = bass_interp.MultiCoreSim(nc, NCORES, num_workers=1)
    for i in range(NCORES):
        for k, v in in_maps[i].items():
            sim.cores[i].tensor(k)[:] = v
    sim.simulate()
    return np.array(sim.cores[0].tensor("out"), dtype=np.float32)


def kernel(**inputs):
    in_maps = make_in_maps(inputs, NCORES)
    try:
        from concourse.bass_utils import run_bass_kernel_spmd

        if "nc" not in _CACHED:
            _CACHED["nc"] = build_nc(NCORES)
        res = run_bass_kernel_spmd(_CACHED["nc"], in_maps, list(range(NCORES)))
        return np.asarray(res.results[0]["out"], dtype=np.float32)
    except Exception:
        return _run_sim(in_maps)


# revision 37
# speedup vs baseline: 1.0443x; 1.0443x over previous
"""BrainGNN forward pass on 8 Trainium2 NeuronCores, data-parallel over batch.

v2 — restructured for speed over the v1 baseline (552us):

  PE cuts (the v1 bottleneck at ~69% busy):
  - augment A@A, msg2: fp32r (fp22-truncated 1-pass matmuls, 4x fp32) with
    N=400 single-span rhs; walrus requires the feeding tiles to be declared
    float32r (DMA / ACT producers round on write).  Sandbox-validated:
    end-to-end rel err ~3e-3 with 13-bit input truncation; HW measured
    1.9e-3 (gate 2e-2).
  - conv2: fp16 hi/lo split, 3 cross terms in 2 matmuls per M-chunk
    (err ~2^-21); the s*keep pooling scale is folded into the per-partition
    ACT scale of the PSUM->SBUF copy.
  - pool score rows (p.hT) folded into the msg matmuls as a 33rd lhsT
    column (per-node q-dots via DVE mult+reduce), removing the fp32
    matvecs.
  - transposed readout: hT is PE-transposed to node-on-partition layout;
    s*keep and the -BIG drop mask are per-partition tensor_scalar scalars,
    the mean is a tiny N=1 ones-matmul (plus a constant -BIG*ndrop
    correction), the max a pair of tensor_reduces.  This kills the
    srep/skrep/krepB row replications of v1.
  - rank colsum matmuls (csp) -> PE transpose of the rank4 column flags.
  - 1/cnt1 pre-folded into alTr columns host-side; Q inherits a recip1
    column scaling that is undone in the hT2 normalization (x cnt1*recip2).

  Emission is software-pipelined: front(b) = loads+conv1+msg1 (PE-dense)
  is emitted before back(b-1) = pools/aug/conv2/msg2 (latency-heavy), so
  the scheduler can fill back's cross-engine stalls with front matmuls.
  Conv combines run ACT(psum->sbuf) -> Pool TT -> DVE reduce; per-graph
  input DMA rides the two HWDGE rings (SP + ACT).

  Exactness notes: pool1 keep set is flip-critical (a boundary flip costs
  ~0.1 rel err); its score path (conv1, msg1, q-dot, compares, transposes)
  is exact fp32 throughout.  pool2 flips cost <2e-3 (sandbox-measured), so
  its scores may ride the fp32r msg2.  tensor_tensor_reduce is avoided
  entirely — it kills the exec unit on some NRT/ucode builds (measured
  here); plain mult+reduce pairs replace it.
"""

import math
import numpy as np

NCORES = 8
B = 64
BL = B // NCORES          # graphs per core
R = 400
KC = 8                    # K_COMM rank of the per-node weight factorization
D1 = 32
D2 = 32
D3 = 512
K1 = math.ceil(0.9 * R)   # 360
K2 = math.ceil(0.9 * K1)  # 324
EPS = 1e-5
BIG = 2.0                 # masked-max offset; |h| < 0.5 validated on CPU
SHIFT = 0.0625            # pool2 masked-score offset (v1-validated)

# 400 = 3*128 + 16 partition chunks
CH = [(0, 128), (128, 128), (256, 128), (384, 16)]
# N splits that keep fp32 matmuls at <=256 free size
SP = [(0, 200), (200, 200)]


def build_nc(n_cores=NCORES, reps=1):
    import concourse.bass as bass
    import concourse.mybir as mybir
    from concourse import tile

    F32 = mybir.dt.float32
    F32R = mybir.dt.float32r
    F16 = mybir.dt.float16
    BF16 = mybir.dt.bfloat16
    AX = mybir.AxisListType
    OP = mybir.AluOpType
    AF = mybir.ActivationFunctionType

    nc = bass.Bass()

    xl = nc.dram_tensor("xl", [BL, R, R], F32, kind="ExternalInput")
    al = nc.dram_tensor("al", [BL, R, R], F32R, kind="ExternalInput")
    alT = nc.dram_tensor("alT", [BL, R, R], F32, kind="ExternalInput")
    r1cl = nc.dram_tensor("r1cl", [BL, 128, 4], F32, kind="ExternalInput")
    cnl = nc.dram_tensor("cnl", [BL, R], F32, kind="ExternalInput")
    w1a = nc.dram_tensor("w1a", [R, KC], F32, kind="ExternalInput")
    bc1 = nc.dram_tensor("bc1", [R, D1 * KC], F32, kind="ExternalInput")
    b1d = nc.dram_tensor("b1d", [D1], F32, kind="ExternalInput")
    p1d = nc.dram_tensor("p1d", [D1], F32, kind="ExternalInput")
    pb1d = nc.dram_tensor("pb1d", [1], F32, kind="ExternalInput")
    p1repd = nc.dram_tensor("p1repd", [128, D1], F32, kind="ExternalInput")
    p2repd = nc.dram_tensor("p2repd", [128, D2], F32, kind="ExternalInput")
    w2a = nc.dram_tensor("w2a", [R, KC], F32, kind="ExternalInput")
    bc2hh = nc.dram_tensor("bc2hh", [2 * D1, D2 * KC], F16, kind="ExternalInput")
    bc2lo = nc.dram_tensor("bc2lo", [D1, D2 * KC], F16, kind="ExternalInput")
    b2d = nc.dram_tensor("b2d", [D2], F32, kind="ExternalInput")
    p2d = nc.dram_tensor("p2d", [D2], F32, kind="ExternalInput")
    pb2d = nc.dram_tensor("pb2d", [1], F32, kind="ExternalInput")
    fc1wd = nc.dram_tensor("fc1wd", [4 * D1, D2], F32, kind="ExternalInput")
    fc1bd = nc.dram_tensor("fc1bd", [D2], F32, kind="ExternalInput")
    g1d = nc.dram_tensor("g1d", [D2], F32, kind="ExternalInput")
    be1d = nc.dram_tensor("be1d", [D2], F32, kind="ExternalInput")
    fc2wd = nc.dram_tensor("fc2wd", [D2, D3], F32, kind="ExternalInput")
    fc2bd = nc.dram_tensor("fc2bd", [D3], F32, kind="ExternalInput")
    g2d = nc.dram_tensor("g2d", [D3], F32, kind="ExternalInput")
    be2d = nc.dram_tensor("be2d", [D3], F32, kind="ExternalInput")
    fc3wd = nc.dram_tensor("fc3wd", [D3, 2], F32, kind="ExternalInput")
    fc3bd = nc.dram_tensor("fc3bd", [2], F32, kind="ExternalInput")
    outd = nc.dram_tensor("out", [B, 2], F32, kind="ExternalOutput")

    from contextlib import ExitStack

    with tile.TileContext(nc) as tc, ExitStack() as es:
        cons = es.enter_context(tc.tile_pool(name="cons", bufs=1))
        work = es.enter_context(tc.tile_pool(name="work", bufs=2))
        dram = es.enter_context(tc.tile_pool(name="dram", bufs=1, space="DRAM"))
        pbig = es.enter_context(tc.tile_pool(name="pbig", bufs=2, space="PSUM"))
        pg = es.enter_context(tc.tile_pool(name="pg", bufs=2, space="PSUM"))
        pacc = es.enter_context(tc.tile_pool(name="pacc", bufs=2, space="PSUM"))
        prep = es.enter_context(tc.tile_pool(name="prep", bufs=2, space="PSUM"))

        # ---------------- constants / weights ----------------
        ones128 = cons.tile([128, 128], F32, tag="ones128")
        nc.vector.memset(ones128[:], 1.0)
        ones_bf = cons.tile([128, D1], BF16, tag="ones_bf")
        nc.vector.memset(ones_bf[:], 1.0)
        ones_r = cons.tile([1, 128], F32, tag="ones_r")
        nc.vector.memset(ones_r[:], 1.0)

        # conv1 weights first: graph 0's matmuls gate the whole pipeline
        bc1t = []
        for c, (o, n) in enumerate(CH):
            tb = cons.tile([n, D1 * KC], F32, tag=f"bc1t{c}")
            nc.sync.dma_start(tb[:], bc1[o:o + n, :])
            bc1t.append(tb)
        a1t, a2t = [], []
        for c, (o, n) in enumerate(CH):
            t = cons.tile([n, KC], F32, tag=f"a1t{c}")
            nc.scalar.dma_start(t[:], w1a[o:o + n, :])
            nc.scalar.activation(t[:], t[:], AF.Relu)
            a1t.append(t)
            t2 = cons.tile([n, KC], F32, tag=f"a2t{c}")
            nc.scalar.dma_start(t2[:], w2a[o:o + n, :])
            nc.scalar.activation(t2[:], t2[:], AF.Relu)
            a2t.append(t2)
        # expanded per-partition combine weights [n, (D1,KC)] (stride-0 free
        # broadcasts materialized once so Pool reads plain tiles)
        a1x, a2x = [], []
        for c, (o, n) in enumerate(CH):
            t = cons.tile([n, D1 * KC], F32, tag=f"a1x{c}")
            nc.vector.tensor_copy(t[:].rearrange("p (o c) -> p o c", c=KC),
                                  a1t[c][:].unsqueeze(1).broadcast_to((n, D1, KC)))
            a1x.append(t)
            t2 = cons.tile([n, D1 * KC], F32, tag=f"a2x{c}")
            nc.vector.tensor_copy(t2[:].rearrange("p (o c) -> p o c", c=KC),
                                  a2t[c][:].unsqueeze(1).broadcast_to((n, D1, KC)))
            a2x.append(t2)

        bc2ht = cons.tile([2 * D1, D2 * KC], F16, tag="bc2ht")
        nc.scalar.dma_start(bc2ht[:], bc2hh[:, :])
        bc2lt = cons.tile([D1, D2 * KC], F16, tag="bc2lt")
        nc.scalar.dma_start(bc2lt[:], bc2lo[:, :])

        def colvec(d, name, nrow):
            t = cons.tile([nrow, 1], F32, tag=name)
            nc.scalar.dma_start(t[:], d[:].unsqueeze(1))
            return t

        b1t = colvec(b1d, "b1t", D1)
        pb1t = colvec(pb1d, "pb1t", 1)
        b2t = colvec(b2d, "b2t", D2)
        pb2t = colvec(pb2d, "pb2t", 1)
        fc1bt = colvec(fc1bd, "fc1bt", D2)
        g1t = colvec(g1d, "g1t", D2)
        be1t = colvec(be1d, "be1t", D2)
        fc3bt = colvec(fc3bd, "fc3bt", 2)

        # p replicated across partitions for the q-dot TTRs (host-shipped)
        p1rep = cons.tile([128, D1], F32, tag="p1rep")
        nc.sync.dma_start(p1rep[:], p1repd[:, :])
        p2rep = cons.tile([128, D2], F32, tag="p2rep")
        nc.sync.dma_start(p2rep[:], p2repd[:, :])

        fc1wt = cons.tile([4 * D1, D2], F32, tag="fc1wt")
        nc.scalar.dma_start(fc1wt[:], fc1wd[:, :])
        fc2wt = cons.tile([D2, D3], F32, tag="fc2wt")
        nc.scalar.dma_start(fc2wt[:], fc2wd[:, :])
        fc2b4 = cons.tile([128, 4], F32, tag="fc2b4")
        nc.sync.dma_start(fc2b4[:], fc2bd[:].rearrange("(c p) -> p c", p=128))
        g24 = cons.tile([128, 4], F32, tag="g24")
        nc.sync.dma_start(g24[:], g2d[:].rearrange("(c p) -> p c", p=128))
        be24 = cons.tile([128, 4], F32, tag="be24")
        nc.sync.dma_start(be24[:], be2d[:].rearrange("(c p) -> p c", p=128))
        fc3wt = cons.tile([128, 8], F32, tag="fc3wt")
        nc.sync.dma_start(fc3wt[:].rearrange("p (c o) -> p c o", o=2),
                          fc3wd[:, :].rearrange("(c p) o -> p c o", p=128))

        I128 = cons.tile([128, 128], F32, tag="I128")
        nc.gpsimd.affine_select(I128[:], ones128[:], pattern=[[-1, 128]],
                                compare_op=OP.is_equal, fill=0.0,
                                base=0, channel_multiplier=1)
        notI = cons.tile([128, 128], F32, tag="notI")
        nc.gpsimd.affine_select(notI[:], ones128[:], pattern=[[-1, 128]],
                                compare_op=OP.not_equal, fill=0.0,
                                base=0, channel_multiplier=1)

        # per-engine fences: per-graph ops depend on one late const per
        # engine, not on dozens of const producers (ISA caps sync waits)
        pfence = prep.tile([1, 4], F32, tag="prep")
        fence_pe = nc.tensor.matmul(pfence[:1, 0:1], I128[:, 0:1], I128[:, 0:1])
        dscr = cons.tile([1, 4], F32, tag="dscr")
        fence_dv1 = nc.vector.tensor_copy(dscr[:1, 0:1], notI[0:1, 0:1])
        fence_dv2 = nc.vector.tensor_copy(dscr[:1, 1:2], a2x[3][0:1, 0:1])
        fences = {"pe": fence_pe, "dv1": fence_dv1, "dv2": fence_dv2}
        first_b = {}

        ztile = cons.tile([128, BL], F32, tag="ztile")
        nc.vector.memset(ztile[:], 0.0)
        eps128 = cons.tile([128, 1], F32, tag="eps128")
        nc.vector.memset(eps128[:], EPS)
        nshift128 = cons.tile([128, 1], F32, tag="nshift128")
        nc.vector.memset(nshift128[:], -SHIFT)

        def mm_f32_split(out_ap, lhsT_ap, rhs_ap):
            # keep each fp32 matmul at N<=256 so walrus doesn't auto-fp32r
            nc.tensor.matmul(out_ap[:, 0:200], lhsT_ap, rhs_ap[:, 0:200])
            nc.tensor.matmul(out_ap[:, 200:400], lhsT_ap, rhs_ap[:, 200:400])

        # ---------------- per-graph pipeline (1-graph software pipeline:
        # front(b) = loads+conv1+msg1 (PE-dense), back(b) = pools/aug/conv2/
        # msg2 (latency-heavy); emitting front(b+1) before back(b) keeps PE
        # fed during back's cross-engine handoffs) ----------------

        def col_from_row(row_t, name):
            pcol = prep.tile([128, 4], F32, tag="prep")
            nc.vector.memset(pcol[:, 3:4], 0.0)   # pad rows (s_col reads all)
            for ic, (io, inn) in enumerate(CH):
                nc.tensor.transpose(pcol[:inn, ic:ic + 1],
                                    row_t[:, io:io + inn], I128[:1, :1])
            col = work.tile([128, 4], F32, tag=name)
            nc.scalar.activation(col[:], pcol[:], AF.Identity)
            return col

        def row_from_col(col_t, name):
            prow = prep.tile([1, R], F32, tag="prep")
            for ic, (io, inn) in enumerate(CH):
                nc.tensor.transpose(prow[:1, io:io + inn],
                                    col_t[:inn, ic:ic + 1], I128[:inn, :inn])
            row = work.tile([1, R], F32, tag=name)
            nc.vector.tensor_copy(row[:], prow[:])
            return row

        def rank_keep(row_t, col_t, kthresh, kname, want_row=True):
            """keep_col [128,4] (+ keep_row [1,R]) f32 from score row/col."""
            srep = pbig.tile([128, R], F32, tag="pT")
            nc.tensor.matmul(srep[:, 0:200], ones_r[:], row_t[:, 0:200])
            nc.tensor.matmul(srep[:, 200:400], ones_r[:], row_t[:, 200:400])
            rank4 = work.tile([128, 4], F32, tag=f"{kname}_rk")
            nc.vector.memset(rank4[:, 3:4], 999.0)   # pad rows lose the rank
            for ic, (io, inn) in enumerate(CH):
                cmp = work.tile([128, R], BF16, tag="cmp")
                nc.vector.tensor_scalar(cmp[:inn, :], srep[:inn, :],
                                        col_t[:inn, ic:ic + 1],
                                        0.0, op0=OP.is_gt, op1=OP.add,
                                        accum_out=rank4[:inn, ic:ic + 1])
            keep_col = work.tile([128, 4], F32, tag=f"{kname}_col")
            nc.vector.tensor_scalar(keep_col[:], rank4[:], kthresh, None,
                                    op0=OP.is_lt)
            keep_row = row_from_col(keep_col, f"{kname}_row") if want_row else None
            return keep_col, keep_row

        def readout(b, hT_t, skk_col, bB_col, kdiv, ndrop, zoff):
            # transposed masked readout: hkT[n,d] = hT[d,n]*sk[n] for
            # kept, -BIG for dropped; max via TRs, mean via ones-matmul
            tp = prep.tile([128, 128], F32, tag="prep")
            for ic, (io, inn) in enumerate(CH):
                nc.tensor.transpose(tp[:inn, 32 * ic:32 * ic + D1],
                                    hT_t[:, io:io + inn], I128[:D1, :D1])
            hkT = work.tile([128, 128], F32, tag="hkT")
            # chunk-3 pad rows (nodes 400..511) must lose the max and cancel
            # in the mean correction: fill the whole block, TS rewrites [:16]
            nc.vector.memset(hkT[:, 96:128], -BIG)
            for ic, (io, inn) in enumerate(CH):
                nc.vector.tensor_scalar(hkT[:inn, 32 * ic:32 * ic + D1],
                                        tp[:inn, 32 * ic:32 * ic + D1],
                                        skk_col[:inn, ic:ic + 1],
                                        bB_col[:inn, ic:ic + 1],
                                        op0=OP.mult, op1=OP.add)
            mred = prep.tile([D1, 1], F32, tag="prep")
            for ic, (io, inn) in enumerate(CH):
                nc.tensor.matmul(mred[:], hkT[:inn, 32 * ic:32 * ic + D1],
                                 ones128[:inn, 0:1],
                                 start=(ic == 0), stop=(ic == 3))
            nc.vector.tensor_scalar(ztile[zoff + D1:zoff + 2 * D1, b:b + 1],
                                    mred[:], 1.0 / kdiv, BIG * ndrop / kdiv,
                                    op0=OP.mult, op1=OP.add)
            m1 = work.tile([128, D1], F32, tag="m1")
            nc.vector.tensor_reduce(m1[:],
                                    hkT[:].rearrange("p (c o) -> p o c", c=4),
                                    axis=AX.X, op=OP.max)
            mtr = prep.tile([D1, 128], F32, tag="prep")
            nc.tensor.transpose(mtr[:], m1[:], I128[:, :])
            nc.vector.tensor_reduce(ztile[zoff:zoff + D1, b:b + 1], mtr[:],
                                    axis=AX.X, op=OP.max)

        def front(b):
            # input loads on the two HWDGE rings (SP + ACT); Pool kept free
            xt, at, Ts = [], [], []
            for c, (o, n) in enumerate(CH):
                t = work.tile([n, R], F32, tag=f"xt{c}")
                nc.sync.dma_start(t[:], xl[b, o:o + n, :])
                xt.append(t)
                t = work.tile([n, R], F32R, tag=f"at{c}")
                nc.scalar.dma_start(t[:], al[b, o:o + n, :])
                at.append(t)
                t = work.tile([n, R], F32, tag=f"Ts{c}")
                (nc.sync if c % 2 else nc.scalar).dma_start(t[:], alT[b, o:o + n, :])
                Ts.append(t)
            r1c = work.tile([128, 4], F32, tag="r1c")
            nc.sync.dma_start(r1c[:], r1cl[b])
            c1rep = work.tile([D2, R], F32, tag="c1rep")
            nc.sync.dma_start(c1rep[:],
                              cnl[b, :].unsqueeze(0).broadcast_to((D2, R)))

            # conv1 (fp32): G = h @ [B_c], combine on ACT->Pool->DVE
            ht1e = []
            for mc, (mo, mn) in enumerate(CH):
                gp = pg.tile([mn, D1 * KC], F32, tag="pG")
                for dc, (do, dn) in enumerate(CH):
                    mm = nc.tensor.matmul(gp[:], xt[dc][:, mo:mo + mn], bc1t[dc][:],
                                          start=(dc == 0), stop=(dc == 3))
                    first_b.setdefault("g1", mm)
                gs = work.tile([mn, D1 * KC], F32, tag="gs")
                aa = nc.scalar.activation(gs[:], gp[:], AF.Identity)
                first_b.setdefault("gs", aa)
                t = work.tile([mn, D1 + 1], F32, tag=f"ht1_{mc}")
                prod = work.tile([mn, D1 * KC], F32, tag="prod")
                pp = nc.gpsimd.tensor_tensor(prod[:], gs[:], a1x[mc][:], op=OP.mult)
                first_b.setdefault("prod", pp)
                rr = nc.vector.tensor_reduce(t[:, 0:D1],
                                             prod[:].rearrange("p (o c) -> p o c", c=KC),
                                             axis=AX.X, op=OP.add)
                first_b.setdefault("red", rr)
                qd = work.tile([mn, D1], F32, tag="qd")
                nc.vector.tensor_tensor(qd[:], t[:, 0:D1], p1rep[:mn, :],
                                        op=OP.mult)
                nc.vector.tensor_reduce(t[:, D1:D1 + 1], qd[:], axis=AX.X,
                                        op=OP.add)
                ht1e.append(t)

            # msg1 (fp32): rows 0..31 = hT1 pre-bias, row 32 = score row
            msgp = pacc.tile([D1 + 1, R], F32, tag="pacc")
            for jc, (jo, jn) in enumerate(CH):
                for so, sn in SP:
                    nc.tensor.matmul(msgp[:, so:so + sn], ht1e[jc][:],
                                     Ts[jc][:, so:so + sn],
                                     start=(jc == 0 and so == 0),
                                     stop=(jc == 3 and so == 200))
            return dict(at=at, Ts=Ts, r1c=r1c, c1rep=c1rep, msgp=msgp)

        def back(b, st):
            at, Ts, r1c, c1rep, msgp = (st["at"], st["Ts"], st["r1c"],
                                        st["c1rep"], st["msgp"])
            hT1 = work.tile([D1, R], F32, tag="hT1")
            nc.scalar.activation(hT1[:], msgp[0:D1, :], AF.Identity, bias=b1t[:])
            pre_row = work.tile([1, R], F32, tag="pre_row")
            nc.scalar.activation(pre_row[:], msgp[D1:D1 + 1, :], AF.Identity,
                                 bias=pb1t[:])

            pre_col = col_from_row(pre_row, "pre_col")
            keep_col, keep_row = rank_keep(pre_row, pre_col, K1 - 0.5, "k1")

            s_col = work.tile([128, 4], F32, tag="s_col")
            nc.scalar.activation(s_col[:], pre_col[:], AF.Sigmoid)
            skk_col = work.tile([128, 4], F32, tag="skk_col")
            nc.vector.tensor_tensor(skk_col[:], s_col[:], keep_col[:], op=OP.mult)
            bB_col = work.tile([128, 4], F32, tag="bB_col")
            nc.vector.tensor_scalar(bB_col[:], keep_col[:], BIG, -BIG,
                                    op0=OP.mult, op1=OP.add)
            readout(b, hT1, skk_col, bB_col, K1, R - K1, 0)

            # augment (fp32r): QR = (T+I) S (T+I) * diag(recip1)
            kr1 = work.tile([128, 4], F32, tag="kr1")
            nc.vector.tensor_tensor(kr1[:], keep_col[:], r1c[:], op=OP.mult)
            notk_rowb = work.tile([1, R], BF16, tag="notk_rowb")
            nc.vector.tensor_scalar(notk_rowb[:], keep_row[:], 0.5, None,
                                    op0=OP.is_lt)
            wTr = []
            for jc, (jo, jn) in enumerate(CH):
                wt = work.tile([jn, R], F32R, tag=f"wTr{jc}")
                nc.scalar.activation(wt[:], Ts[jc][:], AF.Identity,
                                     scale=keep_col[:jn, jc:jc + 1])
                wTr.append(wt)
            Qs = []
            cnt2p = pacc.tile([D2, R], F32, tag="pacc")
            for uc, (uo, un) in enumerate(CH):
                qp = pbig.tile([un, R], F32, tag="pT")
                for jc, (jo, jn) in enumerate(CH):
                    nc.tensor.matmul(qp[:], at[jc][:, uo:uo + un], wTr[jc][:],
                                     start=(jc == 0), stop=(jc == 3))
                nc.vector.tensor_tensor(qp[:, uo:uo + un], qp[:, uo:uo + un],
                                        notI[:un, :un], op=OP.mult)
                nc.vector.scalar_tensor_tensor(qp[:, uo:uo + un], I128[:un, :un],
                                               kr1[:un, uc:uc + 1],
                                               qp[:, uo:uo + un],
                                               op0=OP.mult, op1=OP.add)
                ind2 = work.tile([un, R], BF16, tag="ind")
                nc.vector.tensor_scalar(ind2[:], qp[:], 0.0,
                                        keep_col[:un, uc:uc + 1],
                                        op0=OP.is_gt, op1=OP.mult)
                nc.tensor.matmul(cnt2p[:], ones_bf[:un, :D2], ind2[:],
                                 start=(uc == 0), stop=False)
                q = work.tile([un, R], F32R, tag=f"Qs{uc}")
                nc.scalar.activation(q[:], qp[:], AF.Identity)
                Qs.append(q)
            # dropped rows: count += 1 so the reciprocal stays finite
            nc.tensor.matmul(cnt2p[:], ones_bf[:1, :D2], notk_rowb[:],
                             start=False, stop=True)
            rec2 = work.tile([D2, R], F32, tag="rec2")
            nc.vector.reciprocal(rec2[:], cnt2p[:])
            f2 = work.tile([D2, R], F32, tag="f2")
            nc.vector.tensor_tensor(f2[:], rec2[:], c1rep[:], op=OP.mult)

            # conv2 (fp16 3-term): hT1 split hi/lo; s*keep folded into the
            # per-partition scale of the PSUM->SBUF copy
            hk16 = work.tile([2 * D1, R], F16, tag="hk16")
            nc.vector.tensor_copy(hk16[0:D1, :], hT1[:])
            nc.vector.tensor_tensor(hk16[D1:2 * D1, :], hT1[:], hk16[0:D1, :],
                                    op=OP.subtract)
            ht2e = []
            for mc, (mo, mn) in enumerate(CH):
                gp2 = pg.tile([mn, D2 * KC], F32, tag="pG")
                nc.tensor.matmul(gp2[:], hk16[:, mo:mo + mn], bc2ht[:],
                                 start=True, stop=False)
                nc.tensor.matmul(gp2[:], hk16[0:D1, mo:mo + mn], bc2lt[:],
                                 start=False, stop=True)
                gs2 = work.tile([mn, D2 * KC], F32, tag="gs")
                nc.scalar.activation(gs2[:], gp2[:], AF.Identity,
                                     scale=skk_col[:mn, mc:mc + 1])
                prod2 = work.tile([mn, D2 * KC], F32, tag="prod")
                nc.gpsimd.tensor_tensor(prod2[:], gs2[:], a2x[mc][:], op=OP.mult)
                t = work.tile([mn, D2 + 1], F32R, tag=f"ht1_{mc}")
                tf = work.tile([mn, D2], F32, tag="tf")
                nc.vector.tensor_reduce(tf[:],
                                        prod2[:].rearrange("p (o c) -> p o c", c=KC),
                                        axis=AX.X, op=OP.add)
                qd2 = work.tile([mn, D2], F32, tag="qd")
                nc.vector.tensor_tensor(qd2[:], tf[:], p2rep[:mn, :], op=OP.mult)
                with nc.allow_low_precision("fp32r feed of the msg2 matmul; "
                                            "sandbox-validated"):
                    nc.vector.tensor_copy(t[:, 0:D2], tf[:])
                    nc.vector.tensor_reduce(t[:, D2:D2 + 1], qd2[:], axis=AX.X,
                                            op=OP.add)
                ht2e.append(t)

            # msg2 (fp32r)
            msg2p = pacc.tile([D2 + 1, R], F32, tag="pacc")
            for jc, (jo, jn) in enumerate(CH):
                nc.tensor.matmul(msg2p[:], ht2e[jc][:], Qs[jc][:],
                                 start=(jc == 0), stop=(jc == 3))
            hT2m = work.tile([D2, R], F32, tag="hT2m")
            nc.vector.tensor_tensor(hT2m[:], msg2p[0:D2, :], f2[:], op=OP.mult)
            hT2 = work.tile([D2, R], F32, tag="hT1")
            nc.scalar.activation(hT2[:], hT2m[:], AF.Identity, bias=b2t[:])
            pre2r = work.tile([1, R], F32, tag="pre2r")
            nc.vector.tensor_tensor(pre2r[:], msg2p[D2:D2 + 1, :], f2[0:1, :],
                                    op=OP.mult)
            pre2f = work.tile([1, R], F32, tag="pre2f")
            nc.scalar.activation(pre2f[:], pre2r[:], AF.Identity, bias=pb2t[:])

            # pool2 (masked scores; flips here are benign)
            mp_row = work.tile([1, R], F32, tag="mp_row")
            nc.vector.scalar_tensor_tensor(mp_row[:], pre2f[:], SHIFT,
                                           keep_row[:], op0=OP.add, op1=OP.mult)
            mp_col = col_from_row(mp_row, "mp_col")
            keep2_col, _ = rank_keep(mp_row, mp_col, K2 - 0.5, "k2",
                                     want_row=False)
            s2_col = work.tile([128, 4], F32, tag="s_col")
            nc.scalar.activation(s2_col[:], mp_col[:], AF.Sigmoid,
                                 bias=nshift128[:])
            skk2_col = work.tile([128, 4], F32, tag="skk_col")
            nc.vector.tensor_tensor(skk2_col[:], s2_col[:], keep2_col[:],
                                    op=OP.mult)
            bB2_col = work.tile([128, 4], F32, tag="bB_col")
            nc.vector.tensor_scalar(bB2_col[:], keep2_col[:], BIG, -BIG,
                                    op0=OP.mult, op1=OP.add)
            readout(b, hT2, skk2_col, bB2_col, K2, R - K2, 2 * D1)

        seq = [bb for _ in range(reps) for bb in range(BL)]
        pending = None
        for b in seq:
            st = front(b)
            if pending is not None:
                back(*pending)
            pending = (b, st)
        back(*pending)

        from concourse.tile import add_dep_helper
        for k, tgt in (("g1", "pe"), ("red", "dv1"), ("red", "dv2")):
            if k in first_b:
                add_dep_helper(first_b[k].ins, fences[tgt].ins, sync=False,
                               reason="const fence ordering")

        # ---------------- AllGather + head (redundant on every core) --------
        zloc = dram.tile([128, BL], F32)
        zag = dram.tile([128 * n_cores, BL], F32)
        nc.gpsimd.dma_start(zloc[:], ztile[:])
        if n_cores == 1:
            nc.gpsimd.dma_start(zag[:], zloc[:])
        else:
            import concourse.mybir as _mb
            nc.gpsimd.collective_compute(
                "AllGather",
                _mb.AluOpType.bypass,
                replica_groups=[list(range(n_cores))],
                ins=[zloc[:].opt()],
                outs=[zag[:].opt()],
            )
        ZT = cons.tile([128, B], F32, tag="ZT")
        if n_cores == 1:
            nc.vector.memset(ZT[:], 0.0)
            nc.sync.dma_start(ZT[:, 0:BL], zag[:])
        else:
            nc.sync.dma_start(ZT[:].rearrange("p (c b) -> p c b", b=BL),
                              zag[:].rearrange("(c p) b -> p c b", p=128))

        def bn(y, n, gain, beta):
            mu = cons.tile([n, 1], F32, tag="bn_mu")
            nc.vector.tensor_reduce(mu[:], y[:], axis=AX.X, op=OP.add)
            nc.vector.tensor_scalar(mu[:], mu[:], 1.0 / B, None, op0=OP.mult)
            cen = cons.tile([n, B], F32, tag="bn_cen")
            nc.vector.tensor_scalar(cen[:], y[:], mu[:], None, op0=OP.subtract)
            sq = cons.tile([n, B], F32, tag="bn_sq")
            nc.vector.tensor_tensor(sq[:], cen[:], cen[:], op=OP.mult)
            var = cons.tile([n, 1], F32, tag="bn_var")
            nc.vector.tensor_reduce(var[:], sq[:], axis=AX.X, op=OP.add)
            rstd = cons.tile([n, 1], F32, tag="bn_rstd")
            nc.scalar.activation(rstd[:], var[:], AF.Sqrt, bias=eps128[:n, :],
                                 scale=1.0 / B)
            nc.vector.reciprocal(rstd[:], rstd[:])
            gn = cons.tile([n, 1], F32, tag="bn_gn")
            nc.vector.tensor_tensor(gn[:], rstd[:], gain, op=OP.mult)
            nc.vector.tensor_scalar(y[:], cen[:], gn[:], beta, op0=OP.mult, op1=OP.add)

        y1p = pg.tile([D2, B], F32, tag="pG")
        nc.tensor.matmul(y1p[:], fc1wt[:], ZT[:])
        y1 = cons.tile([D2, B], F32, tag="y1")
        nc.scalar.activation(y1[:], y1p[:], AF.Relu, bias=fc1bt[:])
        bn(y1, D2, g1t[:], be1t[:])

        y3p = pacc.tile([2, B], F32, tag="pacc")
        for mc in range(4):
            y2p = pg.tile([128, B], F32, tag="pG")
            nc.tensor.matmul(y2p[:], fc2wt[:, 128 * mc:128 * (mc + 1)], y1[:])
            y2 = cons.tile([128, B], F32, tag="y2")
            nc.scalar.activation(y2[:], y2p[:], AF.Relu, bias=fc2b4[:, mc:mc + 1])
            bn(y2, 128, g24[:, mc:mc + 1], be24[:, mc:mc + 1])
            nc.tensor.matmul(y3p[:], fc3wt[:, 2 * mc:2 * (mc + 1)], y2[:],
                             start=(mc == 0), stop=(mc == 3))
        y3 = cons.tile([2, B], F32, tag="y3")
        nc.scalar.activation(y3[:], y3p[:], AF.Identity, bias=fc3bt[:])
        nc.sync.dma_start(outd[:, :].rearrange("b o -> o b"), y3[:])

    # Walrus' MM descriptor holds a single sync wait; split multi-waits the
    # same way Bacc.compile does, then populate .instr bytes for extended
    # insts (reciprocal etc).
    import bass_rust as _br
    _br.move_matmul_waits_to_ldweights(nc.m)
    _br.generate_event_semaphores(nc)
    mybir.codegen_inst_isa_subclasses(nc)
    return nc


def make_in_maps(inputs, n_cores=NCORES):
    f32 = np.float32
    f16 = np.float16
    x = np.ascontiguousarray(inputs["x"], dtype=f32)
    adj = np.ascontiguousarray(inputs["adj_w"], dtype=f32)
    p1n = (inputs["p1"] / np.linalg.norm(inputs["p1"])).astype(f32)
    p2n = (inputs["p2"] / np.linalg.norm(inputs["p2"])).astype(f32)
    bc2 = np.ascontiguousarray(
        inputs["W2b"].reshape(KC, D1, D2).transpose(1, 2, 0).reshape(D1, D2 * KC), f32)
    bc2h = bc2.astype(f16)
    bc2l = (bc2 - bc2h.astype(f32)).astype(f16)
    shared = {
        "w1a": np.ascontiguousarray(inputs["W1a"], f32),
        "bc1": np.ascontiguousarray(
            inputs["W1b"].reshape(KC, R, D1).transpose(1, 2, 0).reshape(R, D1 * KC), f32),
        "b1d": np.ascontiguousarray(inputs["b1"], f32),
        "p1d": p1n,
        "pb1d": np.array([np.dot(p1n, inputs["b1"].astype(f32))], f32),
        "p1repd": np.ascontiguousarray(np.tile(p1n, (128, 1))),
        "p2repd": np.ascontiguousarray(np.tile(p2n, (128, 1))),
        "w2a": np.ascontiguousarray(inputs["W2a"], f32),
        "bc2hh": np.ascontiguousarray(np.concatenate([bc2h, bc2h], 0)),
        "bc2lo": np.ascontiguousarray(bc2l),
        "b2d": np.ascontiguousarray(inputs["b2"], f32),
        "p2d": p2n,
        "pb2d": np.array([np.dot(p2n, inputs["b2"].astype(f32))], f32),
        "fc1wd": np.ascontiguousarray(inputs["fc1_w"], f32),
        "fc1bd": np.ascontiguousarray(inputs["fc1_b"], f32),
        "g1d": np.ascontiguousarray(inputs["g1"], f32),
        "be1d": np.ascontiguousarray(inputs["be1"], f32),
        "fc2wd": np.ascontiguousarray(inputs["fc2_w"], f32),
        "fc2bd": np.ascontiguousarray(inputs["fc2_b"], f32),
        "g2d": np.ascontiguousarray(inputs["g2"], f32),
        "be2d": np.ascontiguousarray(inputs["be2"], f32),
        "fc3wd": np.ascontiguousarray(inputs["fc3_w"], f32),
        "fc3bd": np.ascontiguousarray(inputs["fc3_b"], f32),
    }
    cnt = 1.0 + np.asarray(inputs["adj_mask"], bool).sum(-1).astype(f32)
    rcl = (np.float32(1.0) / cnt).astype(f32)
    eye = np.eye(R, dtype=f32)
    BLc = B // n_cores
    maps = []
    for c in range(n_cores):
        m = dict(shared)
        sl = slice(c * BLc, (c + 1) * BLc)
        aI = adj[sl] + eye
        # (A+I)^T with columns j scaled by recip1[j]
        aIT = aI.transpose(0, 2, 1) * rcl[sl][:, None, :]
        m["xl"] = np.ascontiguousarray(x[sl])
        m["al"] = np.ascontiguousarray(aI)
        m["alT"] = np.ascontiguousarray(aIT.astype(f32))
        # recip1 in [128, 4] col-chunk layout
        r1p = np.zeros((BLc, 512), f32)
        r1p[:, :R] = rcl[sl]
        m["r1cl"] = np.ascontiguousarray(r1p.reshape(BLc, 4, 128).transpose(0, 2, 1))
        m["cnl"] = np.ascontiguousarray(cnt[sl])
        maps.append(m)
    return maps


_CACHED = {}


def _run_sim(in_maps):
    # Fallback executor: 8-core CoreSim of the same BIR.
    from concourse import bass_interp

    nc = build_nc(NCORES)
    sim = bass_interp.MultiCoreSim(nc, NCORES, num_workers=1)
    for i in range(NCORES):
        for k, v in in_maps[i].items():
            sim.cores[i].tensor(k)[:] = v
    sim.simulate()
    return np.array(sim.cores[0].tensor("out"), dtype=np.float32)


def kernel(**inputs):
    in_maps = make_in_maps(inputs, NCORES)
    try:
        from concourse.bass_utils import run_bass_kernel_spmd

        if "nc" not in _CACHED:
            _CACHED["nc"] = build_nc(NCORES)
        res = run_bass_kernel_spmd(_CACHED["nc"], in_maps, list(range(NCORES)))
        return np.asarray(res.results[0]["out"], dtype=np.float32)
    except Exception:
        return _run_sim(in_maps)


# revision 46
# speedup vs baseline: 1.0798x; 1.0340x over previous
"""BrainGNN forward pass on 8 Trainium2 NeuronCores, data-parallel over batch.

v2 — restructured for speed over the v1 baseline (552us):

  PE cuts (the v1 bottleneck at ~69% busy):
  - augment A@A, msg2: fp32r (fp22-truncated 1-pass matmuls, 4x fp32) with
    N=400 single-span rhs; walrus requires the feeding tiles to be declared
    float32r (DMA / ACT producers round on write).  Sandbox-validated:
    end-to-end rel err ~3e-3 with 13-bit input truncation; HW measured
    1.9e-3 (gate 2e-2).
  - conv2: fp16 hi/lo split, 3 cross terms in 2 matmuls per M-chunk
    (err ~2^-21); the s*keep pooling scale is folded into the per-partition
    ACT scale of the PSUM->SBUF copy.
  - pool score rows (p.hT) folded into the msg matmuls as a 33rd lhsT
    column (per-node q-dots via DVE mult+reduce), removing the fp32
    matvecs.
  - transposed readout: hT is PE-transposed to node-on-partition layout;
    s*keep and the -BIG drop mask are per-partition tensor_scalar scalars,
    the mean is a tiny N=1 ones-matmul (plus a constant -BIG*ndrop
    correction), the max a pair of tensor_reduces.  This kills the
    srep/skrep/krepB row replications of v1.
  - rank colsum matmuls (csp) -> PE transpose of the rank4 column flags.
  - 1/cnt1 pre-folded into alTr columns host-side; Q inherits a recip1
    column scaling that is undone in the hT2 normalization (x cnt1*recip2).

  Emission is software-pipelined: front(b) = loads+conv1+msg1 (PE-dense)
  is emitted before back(b-1) = pools/aug/conv2/msg2 (latency-heavy), so
  the scheduler can fill back's cross-engine stalls with front matmuls.
  Conv combines run ACT(psum->sbuf) -> Pool TT -> DVE reduce; per-graph
  input DMA rides the two HWDGE rings (SP + ACT).

  Exactness notes: pool1 keep set is flip-critical (a boundary flip costs
  ~0.1 rel err); its score path (conv1, msg1, q-dot, compares, transposes)
  is exact fp32 throughout.  pool2 flips cost <2e-3 (sandbox-measured), so
  its scores may ride the fp32r msg2.  tensor_tensor_reduce is avoided
  entirely — it kills the exec unit on some NRT/ucode builds (measured
  here); plain mult+reduce pairs replace it.
"""

import math
import numpy as np

NCORES = 8
B = 64
BL = B // NCORES          # graphs per core
R = 400
KC = 8                    # K_COMM rank of the per-node weight factorization
D1 = 32
D2 = 32
D3 = 512
K1 = math.ceil(0.9 * R)   # 360
K2 = math.ceil(0.9 * K1)  # 324
EPS = 1e-5
BIG = 2.0                 # masked-max offset; |h| < 0.5 validated on CPU
SHIFT = 0.0625            # pool2 masked-score offset (v1-validated)

# 400 = 3*128 + 16 partition chunks
CH = [(0, 128), (128, 128), (256, 128), (384, 16)]
# N splits that keep fp32 matmuls at <=256 free size
SP = [(0, 200), (200, 200)]


def build_nc(n_cores=NCORES, reps=1):
    import concourse.bass as bass
    import concourse.mybir as mybir
    from concourse import tile

    F32 = mybir.dt.float32
    F32R = mybir.dt.float32r
    F16 = mybir.dt.float16
    BF16 = mybir.dt.bfloat16
    AX = mybir.AxisListType
    OP = mybir.AluOpType
    AF = mybir.ActivationFunctionType

    nc = bass.Bass()

    xl = nc.dram_tensor("xl", [BL, R, R], F32, kind="ExternalInput")
    al = nc.dram_tensor("al", [BL, R, R], F32R, kind="ExternalInput")
    alT = nc.dram_tensor("alT", [BL, R, R], F32, kind="ExternalInput")
    r1cl = nc.dram_tensor("r1cl", [BL, 128, 4], F32, kind="ExternalInput")
    cnl = nc.dram_tensor("cnl", [BL, R], F32, kind="ExternalInput")
    w1a = nc.dram_tensor("w1a", [R, KC], F32, kind="ExternalInput")
    bc1 = nc.dram_tensor("bc1", [R, D1 * KC], F32, kind="ExternalInput")
    b1d = nc.dram_tensor("b1d", [D1], F32, kind="ExternalInput")
    p1d = nc.dram_tensor("p1d", [D1], F32, kind="ExternalInput")
    pb1d = nc.dram_tensor("pb1d", [1], F32, kind="ExternalInput")
    p1repd = nc.dram_tensor("p1repd", [128, D1], F32, kind="ExternalInput")
    p2repd = nc.dram_tensor("p2repd", [128, D2], F32, kind="ExternalInput")
    w2a = nc.dram_tensor("w2a", [R, KC], F32, kind="ExternalInput")
    bc2hh = nc.dram_tensor("bc2hh", [2 * D1, D2 * KC], F16, kind="ExternalInput")
    bc2lo = nc.dram_tensor("bc2lo", [D1, D2 * KC], F16, kind="ExternalInput")
    b2d = nc.dram_tensor("b2d", [D2], F32, kind="ExternalInput")
    p2d = nc.dram_tensor("p2d", [D2], F32, kind="ExternalInput")
    pb2d = nc.dram_tensor("pb2d", [1], F32, kind="ExternalInput")
    fc1wd = nc.dram_tensor("fc1wd", [4 * D1, D2], F32, kind="ExternalInput")
    fc1bd = nc.dram_tensor("fc1bd", [D2], F32, kind="ExternalInput")
    g1d = nc.dram_tensor("g1d", [D2], F32, kind="ExternalInput")
    be1d = nc.dram_tensor("be1d", [D2], F32, kind="ExternalInput")
    fc2wd = nc.dram_tensor("fc2wd", [D2, D3], F32, kind="ExternalInput")
    fc2bd = nc.dram_tensor("fc2bd", [D3], F32, kind="ExternalInput")
    g2d = nc.dram_tensor("g2d", [D3], F32, kind="ExternalInput")
    be2d = nc.dram_tensor("be2d", [D3], F32, kind="ExternalInput")
    fc3wd = nc.dram_tensor("fc3wd", [D3, 2], F32, kind="ExternalInput")
    fc3bd = nc.dram_tensor("fc3bd", [2], F32, kind="ExternalInput")
    outd = nc.dram_tensor("out", [B, 2], F32, kind="ExternalOutput")

    from contextlib import ExitStack

    with tile.TileContext(nc) as tc, ExitStack() as es:
        cons = es.enter_context(tc.tile_pool(name="cons", bufs=1))
        work = es.enter_context(tc.tile_pool(name="work", bufs=2))
        win = es.enter_context(tc.tile_pool(name="win", bufs=2))
        dram = es.enter_context(tc.tile_pool(name="dram", bufs=1, space="DRAM"))
        # bank budget (8): pbig 3 (srep/qp rotation is the hot path),
        # pg 1 (conv psum is drained fast by the ACT copy), pacc 2, prep 2
        pbig = es.enter_context(tc.tile_pool(name="pbig", bufs=3, space="PSUM"))
        pg = es.enter_context(tc.tile_pool(name="pg", bufs=1, space="PSUM"))
        pacc = es.enter_context(tc.tile_pool(name="pacc", bufs=2, space="PSUM"))
        prep = es.enter_context(tc.tile_pool(name="prep", bufs=2, space="PSUM"))

        # ---------------- constants / weights ----------------
        ones128 = cons.tile([128, 128], F32, tag="ones128")
        nc.vector.memset(ones128[:], 1.0)
        ones_bf = cons.tile([128, D1], BF16, tag="ones_bf")
        nc.vector.memset(ones_bf[:], 1.0)
        ones_r = cons.tile([1, 128], F32, tag="ones_r")
        nc.vector.memset(ones_r[:], 1.0)

        # conv1 weights first: graph 0's matmuls gate the whole pipeline
        bc1t = []
        for c, (o, n) in enumerate(CH):
            tb = cons.tile([n, D1 * KC], F32, tag=f"bc1t{c}")
            nc.sync.dma_start(tb[:], bc1[o:o + n, :])
            bc1t.append(tb)
        a1t, a2t = [], []
        for c, (o, n) in enumerate(CH):
            t = cons.tile([n, KC], F32, tag=f"a1t{c}")
            nc.scalar.dma_start(t[:], w1a[o:o + n, :])
            nc.scalar.activation(t[:], t[:], AF.Relu)
            a1t.append(t)
            t2 = cons.tile([n, KC], F32, tag=f"a2t{c}")
            nc.scalar.dma_start(t2[:], w2a[o:o + n, :])
            nc.scalar.activation(t2[:], t2[:], AF.Relu)
            a2t.append(t2)
        # expanded per-partition combine weights [n, (D1,KC)] (stride-0 free
        # broadcasts materialized once so Pool reads plain tiles)
        a1x, a2x = [], []
        for c, (o, n) in enumerate(CH):
            t = cons.tile([n, D1 * KC], F32, tag=f"a1x{c}")
            nc.vector.tensor_copy(t[:].rearrange("p (o c) -> p o c", c=KC),
                                  a1t[c][:].unsqueeze(1).broadcast_to((n, D1, KC)))
            a1x.append(t)
            t2 = cons.tile([n, D1 * KC], F32, tag=f"a2x{c}")
            nc.vector.tensor_copy(t2[:].rearrange("p (o c) -> p o c", c=KC),
                                  a2t[c][:].unsqueeze(1).broadcast_to((n, D1, KC)))
            a2x.append(t2)

        bc2ht = cons.tile([2 * D1, D2 * KC], F16, tag="bc2ht")
        nc.scalar.dma_start(bc2ht[:], bc2hh[:, :])
        bc2lt = cons.tile([D1, D2 * KC], F16, tag="bc2lt")
        nc.scalar.dma_start(bc2lt[:], bc2lo[:, :])

        def colvec(d, name, nrow):
            t = cons.tile([nrow, 1], F32, tag=name)
            nc.scalar.dma_start(t[:], d[:].unsqueeze(1))
            return t

        b1t = colvec(b1d, "b1t", D1)
        pb1t = colvec(pb1d, "pb1t", 1)
        b2t = colvec(b2d, "b2t", D2)
        pb2t = colvec(pb2d, "pb2t", 1)
        fc1bt = colvec(fc1bd, "fc1bt", D2)
        g1t = colvec(g1d, "g1t", D2)
        be1t = colvec(be1d, "be1t", D2)
        fc3bt = colvec(fc3bd, "fc3bt", 2)

        # p replicated across partitions for the q-dot TTRs (host-shipped)
        p1rep = cons.tile([128, D1], F32, tag="p1rep")
        nc.sync.dma_start(p1rep[:], p1repd[:, :])
        p2rep = cons.tile([128, D2], F32, tag="p2rep")
        nc.sync.dma_start(p2rep[:], p2repd[:, :])

        fc1wt = cons.tile([4 * D1, D2], F32, tag="fc1wt")
        nc.scalar.dma_start(fc1wt[:], fc1wd[:, :])
        fc2wt = cons.tile([D2, D3], F32, tag="fc2wt")
        nc.scalar.dma_start(fc2wt[:], fc2wd[:, :])
        fc2b4 = cons.tile([128, 4], F32, tag="fc2b4")
        nc.sync.dma_start(fc2b4[:], fc2bd[:].rearrange("(c p) -> p c", p=128))
        g24 = cons.tile([128, 4], F32, tag="g24")
        nc.sync.dma_start(g24[:], g2d[:].rearrange("(c p) -> p c", p=128))
        be24 = cons.tile([128, 4], F32, tag="be24")
        nc.sync.dma_start(be24[:], be2d[:].rearrange("(c p) -> p c", p=128))
        fc3wt = cons.tile([128, 8], F32, tag="fc3wt")
        nc.sync.dma_start(fc3wt[:].rearrange("p (c o) -> p c o", o=2),
                          fc3wd[:, :].rearrange("(c p) o -> p c o", p=128))

        I128 = cons.tile([128, 128], F32, tag="I128")
        nc.gpsimd.affine_select(I128[:], ones128[:], pattern=[[-1, 128]],
                                compare_op=OP.is_equal, fill=0.0,
                                base=0, channel_multiplier=1)
        notI = cons.tile([128, 128], F32, tag="notI")
        nc.gpsimd.affine_select(notI[:], ones128[:], pattern=[[-1, 128]],
                                compare_op=OP.not_equal, fill=0.0,
                                base=0, channel_multiplier=1)

        # per-engine fences: per-graph ops depend on one late const per
        # engine, not on dozens of const producers (ISA caps sync waits)
        pfence = prep.tile([1, 4], F32, tag="prep")
        fence_pe = nc.tensor.matmul(pfence[:1, 0:1], I128[:, 0:1], I128[:, 0:1])
        dscr = cons.tile([1, 4], F32, tag="dscr")
        fence_dv1 = nc.vector.tensor_copy(dscr[:1, 0:1], notI[0:1, 0:1])
        fence_dv2 = nc.vector.tensor_copy(dscr[:1, 1:2], a2x[3][0:1, 0:1])
        fences = {"pe": fence_pe, "dv1": fence_dv1, "dv2": fence_dv2}
        first_b = {}

        ztile = cons.tile([128, BL], F32, tag="ztile")
        nc.vector.memset(ztile[:], 0.0)
        eps128 = cons.tile([128, 1], F32, tag="eps128")
        nc.vector.memset(eps128[:], EPS)
        nshift128 = cons.tile([128, 1], F32, tag="nshift128")
        nc.vector.memset(nshift128[:], -SHIFT)

        def mm_f32_split(out_ap, lhsT_ap, rhs_ap):
            # keep each fp32 matmul at N<=256 so walrus doesn't auto-fp32r
            nc.tensor.matmul(out_ap[:, 0:200], lhsT_ap, rhs_ap[:, 0:200])
            nc.tensor.matmul(out_ap[:, 200:400], lhsT_ap, rhs_ap[:, 200:400])

        # ---------------- per-graph pipeline (1-graph software pipeline:
        # front(b) = loads+conv1+msg1 (PE-dense), back(b) = pools/aug/conv2/
        # msg2 (latency-heavy); emitting front(b+1) before back(b) keeps PE
        # fed during back's cross-engine handoffs) ----------------

        def col_from_row(row_t, name):
            pcol = prep.tile([128, 4], F32, tag="prep")
            nc.vector.memset(pcol[:, 3:4], 0.0)   # pad rows (s_col reads all)
            for ic, (io, inn) in enumerate(CH):
                nc.tensor.transpose(pcol[:inn, ic:ic + 1],
                                    row_t[:, io:io + inn], I128[:1, :1])
            col = work.tile([128, 4], F32, tag=name)
            nc.scalar.activation(col[:], pcol[:], AF.Identity)
            return col

        def row_from_col(col_t, name):
            prow = prep.tile([1, R], F32, tag="prep")
            for ic, (io, inn) in enumerate(CH):
                nc.tensor.transpose(prow[:1, io:io + inn],
                                    col_t[:inn, ic:ic + 1], I128[:inn, :inn])
            row = work.tile([1, R], F32, tag=name)
            nc.vector.tensor_copy(row[:], prow[:])
            return row

        def rank_keep(row_t, col_t, kthresh, kname, want_row=True):
            """keep_col [128,4] (+ keep_row [1,R]) f32 from score row/col."""
            srep = pbig.tile([128, R], F32, tag="pT")
            nc.tensor.matmul(srep[:, 0:200], ones_r[:], row_t[:, 0:200])
            nc.tensor.matmul(srep[:, 200:400], ones_r[:], row_t[:, 200:400])
            rank4 = work.tile([128, 4], F32, tag=f"{kname}_rk")
            nc.vector.memset(rank4[:, 3:4], 999.0)   # pad rows lose the rank
            for ic, (io, inn) in enumerate(CH):
                cmp = work.tile([128, R], BF16, tag="cmp")
                nc.vector.tensor_scalar(cmp[:inn, :], srep[:inn, :],
                                        col_t[:inn, ic:ic + 1],
                                        0.0, op0=OP.is_gt, op1=OP.add,
                                        accum_out=rank4[:inn, ic:ic + 1])
            keep_col = work.tile([128, 4], F32, tag=f"{kname}_col")
            nc.vector.tensor_scalar(keep_col[:], rank4[:], kthresh, None,
                                    op0=OP.is_lt)
            keep_row = row_from_col(keep_col, f"{kname}_row") if want_row else None
            return keep_col, keep_row

        def readout(b, hT_t, skk_col, bB_col, kdiv, ndrop, zoff):
            # transposed masked readout: hkT[n,d] = hT[d,n]*sk[n] for
            # kept, -BIG for dropped; max via TRs, mean via ones-matmul
            tp = prep.tile([128, 128], F32, tag="prep")
            for ic, (io, inn) in enumerate(CH):
                nc.tensor.transpose(tp[:inn, 32 * ic:32 * ic + D1],
                                    hT_t[:, io:io + inn], I128[:D1, :D1])
            hkT = work.tile([128, 128], F32, tag="hkT")
            # chunk-3 pad rows (nodes 400..511) must lose the max and cancel
            # in the mean correction: fill the whole block, TS rewrites [:16]
            nc.vector.memset(hkT[:, 96:128], -BIG)
            for ic, (io, inn) in enumerate(CH):
                nc.vector.tensor_scalar(hkT[:inn, 32 * ic:32 * ic + D1],
                                        tp[:inn, 32 * ic:32 * ic + D1],
                                        skk_col[:inn, ic:ic + 1],
                                        bB_col[:inn, ic:ic + 1],
                                        op0=OP.mult, op1=OP.add)
            mred = prep.tile([D1, 1], F32, tag="prep")
            for ic, (io, inn) in enumerate(CH):
                nc.tensor.matmul(mred[:], hkT[:inn, 32 * ic:32 * ic + D1],
                                 ones128[:inn, 0:1],
                                 start=(ic == 0), stop=(ic == 3))
            nc.vector.tensor_scalar(ztile[zoff + D1:zoff + 2 * D1, b:b + 1],
                                    mred[:], 1.0 / kdiv, BIG * ndrop / kdiv,
                                    op0=OP.mult, op1=OP.add)
            m1 = work.tile([128, D1], F32, tag="m1")
            nc.vector.tensor_reduce(m1[:],
                                    hkT[:].rearrange("p (c o) -> p o c", c=4),
                                    axis=AX.X, op=OP.max)
            mtr = prep.tile([D1, 128], F32, tag="prep")
            nc.tensor.transpose(mtr[:], m1[:], I128[:, :])
            nc.vector.tensor_reduce(ztile[zoff:zoff + D1, b:b + 1], mtr[:],
                                    axis=AX.X, op=OP.max)

        def front(b):
            # input loads on the two HWDGE rings (SP + ACT); Pool kept free
            xt, at, Ts = [], [], []
            for c, (o, n) in enumerate(CH):
                t = win.tile([n, R], F32, tag=f"xt{c}")
                nc.sync.dma_start(t[:], xl[b, o:o + n, :])
                xt.append(t)
                t = win.tile([n, R], F32R, tag=f"at{c}")
                nc.scalar.dma_start(t[:], al[b, o:o + n, :])
                at.append(t)
                t = win.tile([n, R], F32, tag=f"Ts{c}")
                (nc.sync if c % 2 else nc.scalar).dma_start(t[:], alT[b, o:o + n, :])
                Ts.append(t)
            r1c = work.tile([128, 4], F32, tag="r1c")
            nc.sync.dma_start(r1c[:], r1cl[b])
            c1rep = work.tile([D2, R], F32, tag="c1rep")
            nc.sync.dma_start(c1rep[:],
                              cnl[b, :].unsqueeze(0).broadcast_to((D2, R)))

            # conv1 (fp32): G = h @ [B_c], combine on ACT->Pool->DVE
            ht1e = []
            for mc, (mo, mn) in enumerate(CH):
                gp = pg.tile([mn, D1 * KC], F32, tag="pG")
                for dc, (do, dn) in enumerate(CH):
                    mm = nc.tensor.matmul(gp[:], xt[dc][:, mo:mo + mn], bc1t[dc][:],
                                          start=(dc == 0), stop=(dc == 3))
                    first_b.setdefault("g1", mm)
                gs = work.tile([mn, D1 * KC], F32, tag="gs", bufs=4)
                aa = nc.scalar.activation(gs[:], gp[:], AF.Identity)
                first_b.setdefault("gs", aa)
                t = work.tile([mn, D1 + 1], F32, tag=f"ht1_{mc}", bufs=3)
                prod = work.tile([mn, D1 * KC], F32, tag="prod", bufs=4)
                pp = nc.gpsimd.tensor_tensor(prod[:], gs[:], a1x[mc][:], op=OP.mult)
                first_b.setdefault("prod", pp)
                rr = nc.vector.tensor_reduce(t[:, 0:D1],
                                             prod[:].rearrange("p (o c) -> p o c", c=KC),
                                             axis=AX.X, op=OP.add)
                first_b.setdefault("red", rr)
                qd = work.tile([mn, D1], F32, tag="qd")
                nc.vector.tensor_tensor(qd[:], t[:, 0:D1], p1rep[:mn, :],
                                        op=OP.mult)
                nc.vector.tensor_reduce(t[:, D1:D1 + 1], qd[:], axis=AX.X,
                                        op=OP.add)
                ht1e.append(t)

            # msg1 (fp32): rows 0..31 = hT1 pre-bias, row 32 = score row
            msgp = pacc.tile([D1 + 1, R], F32, tag="pacc")
            for jc, (jo, jn) in enumerate(CH):
                for so, sn in SP:
                    nc.tensor.matmul(msgp[:, so:so + sn], ht1e[jc][:],
                                     Ts[jc][:, so:so + sn],
                                     start=(jc == 0 and so == 0),
                                     stop=(jc == 3 and so == 200))
            return dict(at=at, Ts=Ts, r1c=r1c, c1rep=c1rep, msgp=msgp)

        def back(b, st):
            at, Ts, r1c, c1rep, msgp = (st["at"], st["Ts"], st["r1c"],
                                        st["c1rep"], st["msgp"])
            hT1 = work.tile([D1, R], F32, tag="hT1", bufs=3)
            nc.scalar.activation(hT1[:], msgp[0:D1, :], AF.Identity, bias=b1t[:])
            pre_row = work.tile([1, R], F32, tag="pre_row")
            nc.scalar.activation(pre_row[:], msgp[D1:D1 + 1, :], AF.Identity,
                                 bias=pb1t[:])

            pre_col = col_from_row(pre_row, "pre_col")
            keep_col, keep_row = rank_keep(pre_row, pre_col, K1 - 0.5, "k1")

            s_col = work.tile([128, 4], F32, tag="s_col")
            nc.scalar.activation(s_col[:], pre_col[:], AF.Sigmoid)
            skk_col = work.tile([128, 4], F32, tag="skk_col")
            nc.vector.tensor_tensor(skk_col[:], s_col[:], keep_col[:], op=OP.mult)
            bB_col = work.tile([128, 4], F32, tag="bB_col")
            nc.vector.tensor_scalar(bB_col[:], keep_col[:], BIG, -BIG,
                                    op0=OP.mult, op1=OP.add)
            readout(b, hT1, skk_col, bB_col, K1, R - K1, 0)

            # augment (fp32r): QR = (T+I) S (T+I) * diag(recip1)
            kr1 = work.tile([128, 4], F32, tag="kr1")
            nc.vector.tensor_tensor(kr1[:], keep_col[:], r1c[:], op=OP.mult)
            notk_rowb = work.tile([1, R], BF16, tag="notk_rowb")
            nc.vector.tensor_scalar(notk_rowb[:], keep_row[:], 0.5, None,
                                    op0=OP.is_lt)
            wTr = []
            for jc, (jo, jn) in enumerate(CH):
                wt = work.tile([jn, R], F32R, tag=f"wTr{jc}")
                nc.scalar.activation(wt[:], Ts[jc][:], AF.Identity,
                                     scale=keep_col[:jn, jc:jc + 1])
                wTr.append(wt)
            Qs = []
            cnt2p = pacc.tile([D2, R], F32, tag="pacc")
            for uc, (uo, un) in enumerate(CH):
                qp = pbig.tile([un, R], F32, tag="pT")
                for jc, (jo, jn) in enumerate(CH):
                    nc.tensor.matmul(qp[:], at[jc][:, uo:uo + un], wTr[jc][:],
                                     start=(jc == 0), stop=(jc == 3))
                nc.vector.tensor_tensor(qp[:, uo:uo + un], qp[:, uo:uo + un],
                                        notI[:un, :un], op=OP.mult)
                nc.vector.scalar_tensor_tensor(qp[:, uo:uo + un], I128[:un, :un],
                                               kr1[:un, uc:uc + 1],
                                               qp[:, uo:uo + un],
                                               op0=OP.mult, op1=OP.add)
                ind2 = work.tile([un, R], BF16, tag="ind")
                nc.vector.tensor_scalar(ind2[:], qp[:], 0.0,
                                        keep_col[:un, uc:uc + 1],
                                        op0=OP.is_gt, op1=OP.mult)
                nc.tensor.matmul(cnt2p[:], ones_bf[:un, :D2], ind2[:],
                                 start=(uc == 0), stop=False)
                q = work.tile([un, R], F32R, tag=f"Qs{uc}")
                nc.scalar.activation(q[:], qp[:], AF.Identity)
                Qs.append(q)
            # dropped rows: count += 1 so the reciprocal stays finite
            nc.tensor.matmul(cnt2p[:], ones_bf[:1, :D2], notk_rowb[:],
                             start=False, stop=True)
            rec2 = work.tile([D2, R], F32, tag="rec2")
            nc.vector.reciprocal(rec2[:], cnt2p[:])
            f2 = work.tile([D2, R], F32, tag="f2")
            nc.vector.tensor_tensor(f2[:], rec2[:], c1rep[:], op=OP.mult)

            # conv2 (fp16 3-term): hT1 split hi/lo; s*keep folded into the
            # per-partition scale of the PSUM->SBUF copy
            hk16 = work.tile([2 * D1, R], F16, tag="hk16")
            nc.vector.tensor_copy(hk16[0:D1, :], hT1[:])
            nc.vector.tensor_tensor(hk16[D1:2 * D1, :], hT1[:], hk16[0:D1, :],
                                    op=OP.subtract)
            ht2e = []
            for mc, (mo, mn) in enumerate(CH):
                gp2 = pg.tile([mn, D2 * KC], F32, tag="pG")
                nc.tensor.matmul(gp2[:], hk16[:, mo:mo + mn], bc2ht[:],
                                 start=True, stop=False)
                nc.tensor.matmul(gp2[:], hk16[0:D1, mo:mo + mn], bc2lt[:],
                                 start=False, stop=True)
                gs2 = work.tile([mn, D2 * KC], F32, tag="gs", bufs=4)
                nc.scalar.activation(gs2[:], gp2[:], AF.Identity,
                                     scale=skk_col[:mn, mc:mc + 1])
                prod2 = work.tile([mn, D2 * KC], F32, tag="prod", bufs=4)
                nc.gpsimd.tensor_tensor(prod2[:], gs2[:], a2x[mc][:], op=OP.mult)
                t = work.tile([mn, D2 + 1], F32R, tag=f"ht1_{mc}", bufs=3)
                tf = work.tile([mn, D2], F32, tag="tf")
                nc.vector.tensor_reduce(tf[:],
                                        prod2[:].rearrange("p (o c) -> p o c", c=KC),
                                        axis=AX.X, op=OP.add)
                qd2 = work.tile([mn, D2], F32, tag="qd")
                nc.vector.tensor_tensor(qd2[:], tf[:], p2rep[:mn, :], op=OP.mult)
                with nc.allow_low_precision("fp32r feed of the msg2 matmul; "
                                            "sandbox-validated"):
                    nc.vector.tensor_copy(t[:, 0:D2], tf[:])
                    nc.vector.tensor_reduce(t[:, D2:D2 + 1], qd2[:], axis=AX.X,
                                            op=OP.add)
                ht2e.append(t)

            # msg2 (fp32r)
            msg2p = pacc.tile([D2 + 1, R], F32, tag="pacc")
            for jc, (jo, jn) in enumerate(CH):
                nc.tensor.matmul(msg2p[:], ht2e[jc][:], Qs[jc][:],
                                 start=(jc == 0), stop=(jc == 3))
            hT2m = work.tile([D2, R], F32, tag="hT2m")
            nc.vector.tensor_tensor(hT2m[:], msg2p[0:D2, :], f2[:], op=OP.mult)
            hT2 = work.tile([D2, R], F32, tag="hT1", bufs=3)
            nc.scalar.activation(hT2[:], hT2m[:], AF.Identity, bias=b2t[:])
            pre2r = work.tile([1, R], F32, tag="pre2r")
            nc.vector.tensor_tensor(pre2r[:], msg2p[D2:D2 + 1, :], f2[0:1, :],
                                    op=OP.mult)
            pre2f = work.tile([1, R], F32, tag="pre2f")
            nc.scalar.activation(pre2f[:], pre2r[:], AF.Identity, bias=pb2t[:])

            # pool2 (masked scores; flips here are benign)
            mp_row = work.tile([1, R], F32, tag="mp_row")
            nc.vector.scalar_tensor_tensor(mp_row[:], pre2f[:], SHIFT,
                                           keep_row[:], op0=OP.add, op1=OP.mult)
            mp_col = col_from_row(mp_row, "mp_col")
            keep2_col, _ = rank_keep(mp_row, mp_col, K2 - 0.5, "k2",
                                     want_row=False)
            s2_col = work.tile([128, 4], F32, tag="s_col")
            nc.scalar.activation(s2_col[:], mp_col[:], AF.Sigmoid,
                                 bias=nshift128[:])
            skk2_col = work.tile([128, 4], F32, tag="skk_col")
            nc.vector.tensor_tensor(skk2_col[:], s2_col[:], keep2_col[:],
                                    op=OP.mult)
            bB2_col = work.tile([128, 4], F32, tag="bB_col")
            nc.vector.tensor_scalar(bB2_col[:], keep2_col[:], BIG, -BIG,
                                    op0=OP.mult, op1=OP.add)
            readout(b, hT2, skk2_col, bB2_col, K2, R - K2, 2 * D1)

        seq = [bb for _ in range(reps) for bb in range(BL)]
        pending = None
        for b in seq:
            st = front(b)
            if pending is not None:
                back(*pending)
            pending = (b, st)
        back(*pending)

        from concourse.tile import add_dep_helper
        for k, tgt in (("g1", "pe"), ("red", "dv1"), ("red", "dv2")):
            if k in first_b:
                add_dep_helper(first_b[k].ins, fences[tgt].ins, sync=False,
                               reason="const fence ordering")

        # ---------------- AllGather + head (redundant on every core) --------
        zloc = dram.tile([128, BL], F32)
        zag = dram.tile([128 * n_cores, BL], F32)
        nc.gpsimd.dma_start(zloc[:], ztile[:])
        if n_cores == 1:
            nc.gpsimd.dma_start(zag[:], zloc[:])
        else:
            import concourse.mybir as _mb
            nc.gpsimd.collective_compute(
                "AllGather",
                _mb.AluOpType.bypass,
                replica_groups=[list(range(n_cores))],
                ins=[zloc[:].opt()],
                outs=[zag[:].opt()],
            )
        ZT = cons.tile([128, B], F32, tag="ZT")
        if n_cores == 1:
            nc.vector.memset(ZT[:], 0.0)
            nc.sync.dma_start(ZT[:, 0:BL], zag[:])
        else:
            nc.sync.dma_start(ZT[:].rearrange("p (c b) -> p c b", b=BL),
                              zag[:].rearrange("(c p) b -> p c b", p=128))

        def bn(y, n, gain, beta):
            mu = cons.tile([n, 1], F32, tag="bn_mu")
            nc.vector.tensor_reduce(mu[:], y[:], axis=AX.X, op=OP.add)
            nc.vector.tensor_scalar(mu[:], mu[:], 1.0 / B, None, op0=OP.mult)
            cen = cons.tile([n, B], F32, tag="bn_cen")
            nc.vector.tensor_scalar(cen[:], y[:], mu[:], None, op0=OP.subtract)
            sq = cons.tile([n, B], F32, tag="bn_sq")
            nc.vector.tensor_tensor(sq[:], cen[:], cen[:], op=OP.mult)
            var = cons.tile([n, 1], F32, tag="bn_var")
            nc.vector.tensor_reduce(var[:], sq[:], axis=AX.X, op=OP.add)
            rstd = cons.tile([n, 1], F32, tag="bn_rstd")
            nc.scalar.activation(rstd[:], var[:], AF.Sqrt, bias=eps128[:n, :],
                                 scale=1.0 / B)
            nc.vector.reciprocal(rstd[:], rstd[:])
            gn = cons.tile([n, 1], F32, tag="bn_gn")
            nc.vector.tensor_tensor(gn[:], rstd[:], gain, op=OP.mult)
            nc.vector.tensor_scalar(y[:], cen[:], gn[:], beta, op0=OP.mult, op1=OP.add)

        y1p = pg.tile([D2, B], F32, tag="pG")
        nc.tensor.matmul(y1p[:], fc1wt[:], ZT[:])
        y1 = cons.tile([D2, B], F32, tag="y1")
        nc.scalar.activation(y1[:], y1p[:], AF.Relu, bias=fc1bt[:])
        bn(y1, D2, g1t[:], be1t[:])

        y3p = pacc.tile([2, B], F32, tag="pacc")
        for mc in range(4):
            y2p = pg.tile([128, B], F32, tag="pG")
            nc.tensor.matmul(y2p[:], fc2wt[:, 128 * mc:128 * (mc + 1)], y1[:])
            y2 = cons.tile([128, B], F32, tag="y2")
            nc.scalar.activation(y2[:], y2p[:], AF.Relu, bias=fc2b4[:, mc:mc + 1])
            bn(y2, 128, g24[:, mc:mc + 1], be24[:, mc:mc + 1])
            nc.tensor.matmul(y3p[:], fc3wt[:, 2 * mc:2 * (mc + 1)], y2[:],
                             start=(mc == 0), stop=(mc == 3))
        y3 = cons.tile([2, B], F32, tag="y3")
        nc.scalar.activation(y3[:], y3p[:], AF.Identity, bias=fc3bt[:])
        nc.sync.dma_start(outd[:, :].rearrange("b o -> o b"), y3[:])

    # Walrus' MM descriptor holds a single sync wait; split multi-waits the
    # same way Bacc.compile does, then populate .instr bytes for extended
    # insts (reciprocal etc).
    import bass_rust as _br
    _br.move_matmul_waits_to_ldweights(nc.m)
    _br.generate_event_semaphores(nc)
    mybir.codegen_inst_isa_subclasses(nc)
    return nc


def make_in_maps(inputs, n_cores=NCORES):
    f32 = np.float32
    f16 = np.float16
    x = np.ascontiguousarray(inputs["x"], dtype=f32)
    adj = np.ascontiguousarray(inputs["adj_w"], dtype=f32)
    p1n = (inputs["p1"] / np.linalg.norm(inputs["p1"])).astype(f32)
    p2n = (inputs["p2"] / np.linalg.norm(inputs["p2"])).astype(f32)
    bc2 = np.ascontiguousarray(
        inputs["W2b"].reshape(KC, D1, D2).transpose(1, 2, 0).reshape(D1, D2 * KC), f32)
    bc2h = bc2.astype(f16)
    bc2l = (bc2 - bc2h.astype(f32)).astype(f16)
    shared = {
        "w1a": np.ascontiguousarray(inputs["W1a"], f32),
        "bc1": np.ascontiguousarray(
            inputs["W1b"].reshape(KC, R, D1).transpose(1, 2, 0).reshape(R, D1 * KC), f32),
        "b1d": np.ascontiguousarray(inputs["b1"], f32),
        "p1d": p1n,
        "pb1d": np.array([np.dot(p1n, inputs["b1"].astype(f32))], f32),
        "p1repd": np.ascontiguousarray(np.tile(p1n, (128, 1))),
        "p2repd": np.ascontiguousarray(np.tile(p2n, (128, 1))),
        "w2a": np.ascontiguousarray(inputs["W2a"], f32),
        "bc2hh": np.ascontiguousarray(np.concatenate([bc2h, bc2h], 0)),
        "bc2lo": np.ascontiguousarray(bc2l),
        "b2d": np.ascontiguousarray(inputs["b2"], f32),
        "p2d": p2n,
        "pb2d": np.array([np.dot(p2n, inputs["b2"].astype(f32))], f32),
        "fc1wd": np.ascontiguousarray(inputs["fc1_w"], f32),
        "fc1bd": np.ascontiguousarray(inputs["fc1_b"], f32),
        "g1d": np.ascontiguousarray(inputs["g1"], f32),
        "be1d": np.ascontiguousarray(inputs["be1"], f32),
        "fc2wd": np.ascontiguousarray(inputs["fc2_w"], f32),
        "fc2bd": np.ascontiguousarray(inputs["fc2_b"], f32),
        "g2d": np.ascontiguousarray(inputs["g2"], f32),
        "be2d": np.ascontiguousarray(inputs["be2"], f32),
        "fc3wd": np.ascontiguousarray(inputs["fc3_w"], f32),
        "fc3bd": np.ascontiguousarray(inputs["fc3_b"], f32),
    }
    cnt = 1.0 + np.asarray(inputs["adj_mask"], bool).sum(-1).astype(f32)
    rcl = (np.float32(1.0) / cnt).astype(f32)
    eye = np.eye(R, dtype=f32)
    BLc = B // n_cores
    maps = []
    for c in range(n_cores):
        m = dict(shared)
        sl = slice(c * BLc, (c + 1) * BLc)
        aI = adj[sl] + eye
        # (A+I)^T with columns j scaled by recip1[j]
        aIT = aI.transpose(0, 2, 1) * rcl[sl][:, None, :]
        m["xl"] = np.ascontiguousarray(x[sl])
        m["al"] = np.ascontiguousarray(aI)
        m["alT"] = np.ascontiguousarray(aIT.astype(f32))
        # recip1 in [128, 4] col-chunk layout
        r1p = np.zeros((BLc, 512), f32)
        r1p[:, :R] = rcl[sl]
        m["r1cl"] = np.ascontiguousarray(r1p.reshape(BLc, 4, 128).transpose(0, 2, 1))
        m["cnl"] = np.ascontiguousarray(cnt[sl])
        maps.append(m)
    return maps


_CACHED = {}


def _run_sim(in_maps):
    # Fallback executor: 8-core CoreSim of the same BIR.
    from concourse import bass_interp

    nc = build_nc(NCORES)
    sim = bass_interp.MultiCoreSim(nc, NCORES, num_workers=1)
    for i in range(NCORES):
        for k, v in in_maps[i].items():
            sim.cores[i].tensor(k)[:] = v
    sim.simulate()
    return np.array(sim.cores[0].tensor("out"), dtype=np.float32)


def kernel(**inputs):
    in_maps = make_in_maps(inputs, NCORES)
    try:
        from concourse.bass_utils import run_bass_kernel_spmd

        if "nc" not in _CACHED:
            _CACHED["nc"] = build_nc(NCORES)
        res = run_bass_kernel_spmd(_CACHED["nc"], in_maps, list(range(NCORES)))
        return np.asarray(res.results[0]["out"], dtype=np.float32)
    except Exception:
        return _run_sim(in_maps)


# revision 47
# speedup vs baseline: 1.0822x; 1.0022x over previous
"""BrainGNN forward pass on 8 Trainium2 NeuronCores, data-parallel over batch.

v2 — restructured for speed over the v1 baseline (552us):

  PE cuts (the v1 bottleneck at ~69% busy):
  - augment A@A, msg2: fp32r (fp22-truncated 1-pass matmuls, 4x fp32) with
    N=400 single-span rhs; walrus requires the feeding tiles to be declared
    float32r (DMA / ACT producers round on write).  Sandbox-validated:
    end-to-end rel err ~3e-3 with 13-bit input truncation; HW measured
    1.9e-3 (gate 2e-2).
  - conv2: fp16 hi/lo split, 3 cross terms in 2 matmuls per M-chunk
    (err ~2^-21); the s*keep pooling scale is folded into the per-partition
    ACT scale of the PSUM->SBUF copy.
  - pool score rows (p.hT) folded into the msg matmuls as a 33rd lhsT
    column (per-node q-dots via DVE mult+reduce), removing the fp32
    matvecs.
  - transposed readout: hT is PE-transposed to node-on-partition layout;
    s*keep and the -BIG drop mask are per-partition tensor_scalar scalars,
    the mean is a tiny N=1 ones-matmul (plus a constant -BIG*ndrop
    correction), the max a pair of tensor_reduces.  This kills the
    srep/skrep/krepB row replications of v1.
  - rank colsum matmuls (csp) -> PE transpose of the rank4 column flags.
  - 1/cnt1 pre-folded into alTr columns host-side; Q inherits a recip1
    column scaling that is undone in the hT2 normalization (x cnt1*recip2).

  Emission is software-pipelined: front(b) = loads+conv1+msg1 (PE-dense)
  is emitted before back(b-1) = pools/aug/conv2/msg2 (latency-heavy), so
  the scheduler can fill back's cross-engine stalls with front matmuls.
  Conv combines run ACT(psum->sbuf) -> Pool TT -> DVE reduce; per-graph
  input DMA rides the two HWDGE rings (SP + ACT).

  Exactness notes: pool1 keep set is flip-critical (a boundary flip costs
  ~0.1 rel err); its score path (conv1, msg1, q-dot, compares, transposes)
  is exact fp32 throughout.  pool2 flips cost <2e-3 (sandbox-measured), so
  its scores may ride the fp32r msg2.  tensor_tensor_reduce is avoided
  entirely — it kills the exec unit on some NRT/ucode builds (measured
  here); plain mult+reduce pairs replace it.
"""

import math
import numpy as np

NCORES = 8
B = 64
BL = B // NCORES          # graphs per core
R = 400
KC = 8                    # K_COMM rank of the per-node weight factorization
D1 = 32
D2 = 32
D3 = 512
K1 = math.ceil(0.9 * R)   # 360
K2 = math.ceil(0.9 * K1)  # 324
EPS = 1e-5
BIG = 2.0                 # masked-max offset; |h| < 0.5 validated on CPU
SHIFT = 0.0625            # pool2 masked-score offset (v1-validated)

# 400 = 3*128 + 16 partition chunks
CH = [(0, 128), (128, 128), (256, 128), (384, 16)]
# N splits that keep fp32 matmuls at <=256 free size
SP = [(0, 200), (200, 200)]


def build_nc(n_cores=NCORES, reps=1):
    import concourse.bass as bass
    import concourse.mybir as mybir
    from concourse import tile

    F32 = mybir.dt.float32
    F32R = mybir.dt.float32r
    F16 = mybir.dt.float16
    BF16 = mybir.dt.bfloat16
    AX = mybir.AxisListType
    OP = mybir.AluOpType
    AF = mybir.ActivationFunctionType

    nc = bass.Bass()

    xl = nc.dram_tensor("xl", [BL, R, R], F32, kind="ExternalInput")
    al = nc.dram_tensor("al", [BL, R, R], F32R, kind="ExternalInput")
    alT = nc.dram_tensor("alT", [BL, R, R], F32, kind="ExternalInput")
    r1cl = nc.dram_tensor("r1cl", [BL, 128, 4], F32, kind="ExternalInput")
    cnl = nc.dram_tensor("cnl", [BL, R], F32, kind="ExternalInput")
    w1a = nc.dram_tensor("w1a", [R, KC], F32, kind="ExternalInput")
    bc1 = nc.dram_tensor("bc1", [R, D1 * KC], F32, kind="ExternalInput")
    b1d = nc.dram_tensor("b1d", [D1], F32, kind="ExternalInput")
    p1d = nc.dram_tensor("p1d", [D1], F32, kind="ExternalInput")
    pb1d = nc.dram_tensor("pb1d", [1], F32, kind="ExternalInput")
    p1repd = nc.dram_tensor("p1repd", [128, D1], F32, kind="ExternalInput")
    p2repd = nc.dram_tensor("p2repd", [128, D2], F32, kind="ExternalInput")
    w2a = nc.dram_tensor("w2a", [R, KC], F32, kind="ExternalInput")
    bc2hh = nc.dram_tensor("bc2hh", [2 * D1, D2 * KC], F16, kind="ExternalInput")
    bc2lo = nc.dram_tensor("bc2lo", [D1, D2 * KC], F16, kind="ExternalInput")
    b2d = nc.dram_tensor("b2d", [D2], F32, kind="ExternalInput")
    p2d = nc.dram_tensor("p2d", [D2], F32, kind="ExternalInput")
    pb2d = nc.dram_tensor("pb2d", [1], F32, kind="ExternalInput")
    fc1wd = nc.dram_tensor("fc1wd", [4 * D1, D2], F32, kind="ExternalInput")
    fc1bd = nc.dram_tensor("fc1bd", [D2], F32, kind="ExternalInput")
    g1d = nc.dram_tensor("g1d", [D2], F32, kind="ExternalInput")
    be1d = nc.dram_tensor("be1d", [D2], F32, kind="ExternalInput")
    fc2wd = nc.dram_tensor("fc2wd", [D2, D3], F32, kind="ExternalInput")
    fc2bd = nc.dram_tensor("fc2bd", [D3], F32, kind="ExternalInput")
    g2d = nc.dram_tensor("g2d", [D3], F32, kind="ExternalInput")
    be2d = nc.dram_tensor("be2d", [D3], F32, kind="ExternalInput")
    fc3wd = nc.dram_tensor("fc3wd", [D3, 2], F32, kind="ExternalInput")
    fc3bd = nc.dram_tensor("fc3bd", [2], F32, kind="ExternalInput")
    outd = nc.dram_tensor("out", [B, 2], F32, kind="ExternalOutput")

    from contextlib import ExitStack

    with tile.TileContext(nc) as tc, ExitStack() as es:
        cons = es.enter_context(tc.tile_pool(name="cons", bufs=1))
        work = es.enter_context(tc.tile_pool(name="work", bufs=2))
        win = es.enter_context(tc.tile_pool(name="win", bufs=2))
        dram = es.enter_context(tc.tile_pool(name="dram", bufs=1, space="DRAM"))
        # bank budget (8): pbig 3 (srep/qp rotation is the hot path),
        # pg 1 (conv psum is drained fast by the ACT copy), pacc 2, prep 2
        pbig = es.enter_context(tc.tile_pool(name="pbig", bufs=3, space="PSUM"))
        pg = es.enter_context(tc.tile_pool(name="pg", bufs=1, space="PSUM"))
        pacc = es.enter_context(tc.tile_pool(name="pacc", bufs=2, space="PSUM"))
        prep = es.enter_context(tc.tile_pool(name="prep", bufs=2, space="PSUM"))

        # ---------------- constants / weights ----------------
        ones128 = cons.tile([128, 128], F32, tag="ones128")
        nc.vector.memset(ones128[:], 1.0)
        ones_bf = cons.tile([128, D1], BF16, tag="ones_bf")
        nc.vector.memset(ones_bf[:], 1.0)
        ones_r = cons.tile([1, 128], F32, tag="ones_r")
        nc.vector.memset(ones_r[:], 1.0)

        # conv1 weights first: graph 0's matmuls gate the whole pipeline
        bc1t = []
        for c, (o, n) in enumerate(CH):
            tb = cons.tile([n, D1 * KC], F32, tag=f"bc1t{c}")
            nc.sync.dma_start(tb[:], bc1[o:o + n, :])
            bc1t.append(tb)
        a1t, a2t = [], []
        for c, (o, n) in enumerate(CH):
            t = cons.tile([n, KC], F32, tag=f"a1t{c}")
            nc.scalar.dma_start(t[:], w1a[o:o + n, :])
            nc.scalar.activation(t[:], t[:], AF.Relu)
            a1t.append(t)
            t2 = cons.tile([n, KC], F32, tag=f"a2t{c}")
            nc.scalar.dma_start(t2[:], w2a[o:o + n, :])
            nc.scalar.activation(t2[:], t2[:], AF.Relu)
            a2t.append(t2)
        # expanded per-partition combine weights [n, (D1,KC)] (stride-0 free
        # broadcasts materialized once so Pool reads plain tiles)
        a1x, a2x = [], []
        for c, (o, n) in enumerate(CH):
            t = cons.tile([n, D1 * KC], F32, tag=f"a1x{c}")
            nc.vector.tensor_copy(t[:].rearrange("p (o c) -> p o c", c=KC),
                                  a1t[c][:].unsqueeze(1).broadcast_to((n, D1, KC)))
            a1x.append(t)
            t2 = cons.tile([n, D1 * KC], F32, tag=f"a2x{c}")
            nc.vector.tensor_copy(t2[:].rearrange("p (o c) -> p o c", c=KC),
                                  a2t[c][:].unsqueeze(1).broadcast_to((n, D1, KC)))
            a2x.append(t2)

        bc2ht = cons.tile([2 * D1, D2 * KC], F16, tag="bc2ht")
        nc.scalar.dma_start(bc2ht[:], bc2hh[:, :])
        bc2lt = cons.tile([D1, D2 * KC], F16, tag="bc2lt")
        nc.scalar.dma_start(bc2lt[:], bc2lo[:, :])

        def colvec(d, name, nrow):
            t = cons.tile([nrow, 1], F32, tag=name)
            nc.scalar.dma_start(t[:], d[:].unsqueeze(1))
            return t

        b1t = colvec(b1d, "b1t", D1)
        pb1t = colvec(pb1d, "pb1t", 1)
        b2t = colvec(b2d, "b2t", D2)
        pb2t = colvec(pb2d, "pb2t", 1)
        fc1bt = colvec(fc1bd, "fc1bt", D2)
        g1t = colvec(g1d, "g1t", D2)
        be1t = colvec(be1d, "be1t", D2)
        fc3bt = colvec(fc3bd, "fc3bt", 2)

        # p replicated across partitions for the q-dot TTRs (host-shipped)
        p1rep = cons.tile([128, D1], F32, tag="p1rep")
        nc.sync.dma_start(p1rep[:], p1repd[:, :])
        p2rep = cons.tile([128, D2], F32, tag="p2rep")
        nc.sync.dma_start(p2rep[:], p2repd[:, :])

        fc1wt = cons.tile([4 * D1, D2], F32, tag="fc1wt")
        nc.scalar.dma_start(fc1wt[:], fc1wd[:, :])
        fc2wt = cons.tile([D2, D3], F32, tag="fc2wt")
        nc.scalar.dma_start(fc2wt[:], fc2wd[:, :])
        fc2b4 = cons.tile([128, 4], F32, tag="fc2b4")
        nc.sync.dma_start(fc2b4[:], fc2bd[:].rearrange("(c p) -> p c", p=128))
        g24 = cons.tile([128, 4], F32, tag="g24")
        nc.sync.dma_start(g24[:], g2d[:].rearrange("(c p) -> p c", p=128))
        be24 = cons.tile([128, 4], F32, tag="be24")
        nc.sync.dma_start(be24[:], be2d[:].rearrange("(c p) -> p c", p=128))
        fc3wt = cons.tile([128, 8], F32, tag="fc3wt")
        nc.sync.dma_start(fc3wt[:].rearrange("p (c o) -> p c o", o=2),
                          fc3wd[:, :].rearrange("(c p) o -> p c o", p=128))

        I128 = cons.tile([128, 128], F32, tag="I128")
        nc.gpsimd.affine_select(I128[:], ones128[:], pattern=[[-1, 128]],
                                compare_op=OP.is_equal, fill=0.0,
                                base=0, channel_multiplier=1)
        notI = cons.tile([128, 128], F32, tag="notI")
        nc.gpsimd.affine_select(notI[:], ones128[:], pattern=[[-1, 128]],
                                compare_op=OP.not_equal, fill=0.0,
                                base=0, channel_multiplier=1)

        # per-engine fences: per-graph ops depend on one late const per
        # engine, not on dozens of const producers (ISA caps sync waits)
        pfence = prep.tile([1, 4], F32, tag="prep")
        fence_pe = nc.tensor.matmul(pfence[:1, 0:1], I128[:, 0:1], I128[:, 0:1])
        dscr = cons.tile([1, 4], F32, tag="dscr")
        fence_dv1 = nc.vector.tensor_copy(dscr[:1, 0:1], notI[0:1, 0:1])
        fence_dv2 = nc.vector.tensor_copy(dscr[:1, 1:2], a2x[3][0:1, 0:1])
        fences = {"pe": fence_pe, "dv1": fence_dv1, "dv2": fence_dv2}
        first_b = {}

        ztile = cons.tile([128, BL], F32, tag="ztile")
        nc.vector.memset(ztile[:], 0.0)
        eps128 = cons.tile([128, 1], F32, tag="eps128")
        nc.vector.memset(eps128[:], EPS)
        nshift128 = cons.tile([128, 1], F32, tag="nshift128")
        nc.vector.memset(nshift128[:], -SHIFT)

        def mm_f32_split(out_ap, lhsT_ap, rhs_ap):
            # keep each fp32 matmul at N<=256 so walrus doesn't auto-fp32r
            nc.tensor.matmul(out_ap[:, 0:200], lhsT_ap, rhs_ap[:, 0:200])
            nc.tensor.matmul(out_ap[:, 200:400], lhsT_ap, rhs_ap[:, 200:400])

        # ---------------- per-graph pipeline (1-graph software pipeline:
        # front(b) = loads+conv1+msg1 (PE-dense), back(b) = pools/aug/conv2/
        # msg2 (latency-heavy); emitting front(b+1) before back(b) keeps PE
        # fed during back's cross-engine handoffs) ----------------

        def col_from_row(row_t, name):
            pcol = prep.tile([128, 4], F32, tag="prep")
            nc.vector.memset(pcol[:, 3:4], 0.0)   # pad rows (s_col reads all)
            for ic, (io, inn) in enumerate(CH):
                nc.tensor.transpose(pcol[:inn, ic:ic + 1],
                                    row_t[:, io:io + inn], I128[:1, :1])
            col = work.tile([128, 4], F32, tag=name)
            nc.scalar.activation(col[:], pcol[:], AF.Identity)
            return col

        def row_from_col(col_t, name):
            prow = prep.tile([1, R], F32, tag="prep")
            for ic, (io, inn) in enumerate(CH):
                nc.tensor.transpose(prow[:1, io:io + inn],
                                    col_t[:inn, ic:ic + 1], I128[:inn, :inn])
            row = work.tile([1, R], F32, tag=name)
            nc.vector.tensor_copy(row[:], prow[:])
            return row

        def rank_keep(row_t, col_t, kthresh, kname, want_row=True):
            """keep_col [128,4] (+ keep_row [1,R]) f32 from score row/col."""
            srep = pbig.tile([128, R], F32, tag="pT")
            nc.tensor.matmul(srep[:, 0:200], ones_r[:], row_t[:, 0:200])
            nc.tensor.matmul(srep[:, 200:400], ones_r[:], row_t[:, 200:400])
            rank4 = work.tile([128, 4], F32, tag=f"{kname}_rk", bufs=3)
            nc.vector.memset(rank4[:, 3:4], 999.0)   # pad rows lose the rank
            for ic, (io, inn) in enumerate(CH):
                cmp = work.tile([128, R], BF16, tag="cmp", bufs=4)
                nc.vector.tensor_scalar(cmp[:inn, :], srep[:inn, :],
                                        col_t[:inn, ic:ic + 1],
                                        0.0, op0=OP.is_gt, op1=OP.add,
                                        accum_out=rank4[:inn, ic:ic + 1])
            keep_col = work.tile([128, 4], F32, tag=f"{kname}_col")
            nc.vector.tensor_scalar(keep_col[:], rank4[:], kthresh, None,
                                    op0=OP.is_lt)
            keep_row = row_from_col(keep_col, f"{kname}_row") if want_row else None
            return keep_col, keep_row

        def readout(b, hT_t, skk_col, bB_col, kdiv, ndrop, zoff):
            # transposed masked readout: hkT[n,d] = hT[d,n]*sk[n] for
            # kept, -BIG for dropped; max via TRs, mean via ones-matmul
            tp = prep.tile([128, 128], F32, tag="prep")
            for ic, (io, inn) in enumerate(CH):
                nc.tensor.transpose(tp[:inn, 32 * ic:32 * ic + D1],
                                    hT_t[:, io:io + inn], I128[:D1, :D1])
            hkT = work.tile([128, 128], F32, tag="hkT")
            # chunk-3 pad rows (nodes 400..511) must lose the max and cancel
            # in the mean correction: fill the whole block, TS rewrites [:16]
            nc.vector.memset(hkT[:, 96:128], -BIG)
            for ic, (io, inn) in enumerate(CH):
                nc.vector.tensor_scalar(hkT[:inn, 32 * ic:32 * ic + D1],
                                        tp[:inn, 32 * ic:32 * ic + D1],
                                        skk_col[:inn, ic:ic + 1],
                                        bB_col[:inn, ic:ic + 1],
                                        op0=OP.mult, op1=OP.add)
            mred = prep.tile([D1, 1], F32, tag="prep")
            for ic, (io, inn) in enumerate(CH):
                nc.tensor.matmul(mred[:], hkT[:inn, 32 * ic:32 * ic + D1],
                                 ones128[:inn, 0:1],
                                 start=(ic == 0), stop=(ic == 3))
            nc.vector.tensor_scalar(ztile[zoff + D1:zoff + 2 * D1, b:b + 1],
                                    mred[:], 1.0 / kdiv, BIG * ndrop / kdiv,
                                    op0=OP.mult, op1=OP.add)
            m1 = work.tile([128, D1], F32, tag="m1")
            nc.vector.tensor_reduce(m1[:],
                                    hkT[:].rearrange("p (c o) -> p o c", c=4),
                                    axis=AX.X, op=OP.max)
            mtr = prep.tile([D1, 128], F32, tag="prep")
            nc.tensor.transpose(mtr[:], m1[:], I128[:, :])
            nc.vector.tensor_reduce(ztile[zoff:zoff + D1, b:b + 1], mtr[:],
                                    axis=AX.X, op=OP.max)

        def front(b):
            # input loads on the two HWDGE rings (SP + ACT); Pool kept free
            xt, at, Ts = [], [], []
            for c, (o, n) in enumerate(CH):
                t = win.tile([n, R], F32, tag=f"xt{c}")
                nc.sync.dma_start(t[:], xl[b, o:o + n, :])
                xt.append(t)
                t = win.tile([n, R], F32R, tag=f"at{c}")
                nc.scalar.dma_start(t[:], al[b, o:o + n, :])
                at.append(t)
                t = win.tile([n, R], F32, tag=f"Ts{c}")
                (nc.sync if c % 2 else nc.scalar).dma_start(t[:], alT[b, o:o + n, :])
                Ts.append(t)
            r1c = work.tile([128, 4], F32, tag="r1c")
            nc.sync.dma_start(r1c[:], r1cl[b])
            c1rep = work.tile([D2, R], F32, tag="c1rep")
            nc.sync.dma_start(c1rep[:],
                              cnl[b, :].unsqueeze(0).broadcast_to((D2, R)))

            # conv1 (fp32): G = h @ [B_c], combine on ACT->Pool->DVE
            ht1e = []
            for mc, (mo, mn) in enumerate(CH):
                gp = pg.tile([mn, D1 * KC], F32, tag="pG")
                for dc, (do, dn) in enumerate(CH):
                    mm = nc.tensor.matmul(gp[:], xt[dc][:, mo:mo + mn], bc1t[dc][:],
                                          start=(dc == 0), stop=(dc == 3))
                    first_b.setdefault("g1", mm)
                gs = work.tile([mn, D1 * KC], F32, tag="gs", bufs=4)
                aa = nc.scalar.activation(gs[:], gp[:], AF.Identity)
                first_b.setdefault("gs", aa)
                t = work.tile([mn, D1 + 1], F32, tag=f"ht1_{mc}", bufs=3)
                prod = work.tile([mn, D1 * KC], F32, tag="prod", bufs=4)
                pp = nc.gpsimd.tensor_tensor(prod[:], gs[:], a1x[mc][:], op=OP.mult)
                first_b.setdefault("prod", pp)
                rr = nc.vector.tensor_reduce(t[:, 0:D1],
                                             prod[:].rearrange("p (o c) -> p o c", c=KC),
                                             axis=AX.X, op=OP.add)
                first_b.setdefault("red", rr)
                qd = work.tile([mn, D1], F32, tag="qd")
                nc.vector.tensor_tensor(qd[:], t[:, 0:D1], p1rep[:mn, :],
                                        op=OP.mult)
                nc.vector.tensor_reduce(t[:, D1:D1 + 1], qd[:], axis=AX.X,
                                        op=OP.add)
                ht1e.append(t)

            # msg1 (fp32): rows 0..31 = hT1 pre-bias, row 32 = score row
            msgp = pacc.tile([D1 + 1, R], F32, tag="pacc")
            for jc, (jo, jn) in enumerate(CH):
                for so, sn in SP:
                    nc.tensor.matmul(msgp[:, so:so + sn], ht1e[jc][:],
                                     Ts[jc][:, so:so + sn],
                                     start=(jc == 0 and so == 0),
                                     stop=(jc == 3 and so == 200))
            return dict(at=at, Ts=Ts, r1c=r1c, c1rep=c1rep, msgp=msgp)

        def back(b, st):
            at, Ts, r1c, c1rep, msgp = (st["at"], st["Ts"], st["r1c"],
                                        st["c1rep"], st["msgp"])
            hT1 = work.tile([D1, R], F32, tag="hT1", bufs=3)
            nc.scalar.activation(hT1[:], msgp[0:D1, :], AF.Identity, bias=b1t[:])
            pre_row = work.tile([1, R], F32, tag="pre_row")
            nc.scalar.activation(pre_row[:], msgp[D1:D1 + 1, :], AF.Identity,
                                 bias=pb1t[:])

            pre_col = col_from_row(pre_row, "pre_col")
            keep_col, keep_row = rank_keep(pre_row, pre_col, K1 - 0.5, "k1")

            s_col = work.tile([128, 4], F32, tag="s_col")
            nc.scalar.activation(s_col[:], pre_col[:], AF.Sigmoid)
            skk_col = work.tile([128, 4], F32, tag="skk_col")
            nc.vector.tensor_tensor(skk_col[:], s_col[:], keep_col[:], op=OP.mult)
            bB_col = work.tile([128, 4], F32, tag="bB_col")
            nc.vector.tensor_scalar(bB_col[:], keep_col[:], BIG, -BIG,
                                    op0=OP.mult, op1=OP.add)
            readout(b, hT1, skk_col, bB_col, K1, R - K1, 0)

            # augment (fp32r): QR = (T+I) S (T+I) * diag(recip1)
            kr1 = work.tile([128, 4], F32, tag="kr1")
            nc.vector.tensor_tensor(kr1[:], keep_col[:], r1c[:], op=OP.mult)
            notk_rowb = work.tile([1, R], BF16, tag="notk_rowb")
            nc.vector.tensor_scalar(notk_rowb[:], keep_row[:], 0.5, None,
                                    op0=OP.is_lt)
            wTr = []
            for jc, (jo, jn) in enumerate(CH):
                wt = work.tile([jn, R], F32R, tag=f"wTr{jc}", bufs=3)
                nc.scalar.activation(wt[:], Ts[jc][:], AF.Identity,
                                     scale=keep_col[:jn, jc:jc + 1])
                wTr.append(wt)
            Qs = []
            cnt2p = pacc.tile([D2, R], F32, tag="pacc")
            for uc, (uo, un) in enumerate(CH):
                qp = pbig.tile([un, R], F32, tag="pT")
                for jc, (jo, jn) in enumerate(CH):
                    nc.tensor.matmul(qp[:], at[jc][:, uo:uo + un], wTr[jc][:],
                                     start=(jc == 0), stop=(jc == 3))
                nc.vector.tensor_tensor(qp[:, uo:uo + un], qp[:, uo:uo + un],
                                        notI[:un, :un], op=OP.mult)
                nc.vector.scalar_tensor_tensor(qp[:, uo:uo + un], I128[:un, :un],
                                               kr1[:un, uc:uc + 1],
                                               qp[:, uo:uo + un],
                                               op0=OP.mult, op1=OP.add)
                ind2 = work.tile([un, R], BF16, tag="ind", bufs=4)
                nc.vector.tensor_scalar(ind2[:], qp[:], 0.0,
                                        keep_col[:un, uc:uc + 1],
                                        op0=OP.is_gt, op1=OP.mult)
                nc.tensor.matmul(cnt2p[:], ones_bf[:un, :D2], ind2[:],
                                 start=(uc == 0), stop=False)
                q = work.tile([un, R], F32R, tag=f"Qs{uc}")
                nc.scalar.activation(q[:], qp[:], AF.Identity)
                Qs.append(q)
            # dropped rows: count += 1 so the reciprocal stays finite
            nc.tensor.matmul(cnt2p[:], ones_bf[:1, :D2], notk_rowb[:],
                             start=False, stop=True)
            rec2 = work.tile([D2, R], F32, tag="rec2")
            nc.vector.reciprocal(rec2[:], cnt2p[:])
            f2 = work.tile([D2, R], F32, tag="f2")
            nc.vector.tensor_tensor(f2[:], rec2[:], c1rep[:], op=OP.mult)

            # conv2 (fp16 3-term): hT1 split hi/lo; s*keep folded into the
            # per-partition scale of the PSUM->SBUF copy
            hk16 = work.tile([2 * D1, R], F16, tag="hk16")
            nc.vector.tensor_copy(hk16[0:D1, :], hT1[:])
            nc.vector.tensor_tensor(hk16[D1:2 * D1, :], hT1[:], hk16[0:D1, :],
                                    op=OP.subtract)
            ht2e = []
            for mc, (mo, mn) in enumerate(CH):
                gp2 = pg.tile([mn, D2 * KC], F32, tag="pG")
                nc.tensor.matmul(gp2[:], hk16[:, mo:mo + mn], bc2ht[:],
                                 start=True, stop=False)
                nc.tensor.matmul(gp2[:], hk16[0:D1, mo:mo + mn], bc2lt[:],
                                 start=False, stop=True)
                gs2 = work.tile([mn, D2 * KC], F32, tag="gs", bufs=4)
                nc.scalar.activation(gs2[:], gp2[:], AF.Identity,
                                     scale=skk_col[:mn, mc:mc + 1])
                prod2 = work.tile([mn, D2 * KC], F32, tag="prod", bufs=4)
                nc.gpsimd.tensor_tensor(prod2[:], gs2[:], a2x[mc][:], op=OP.mult)
                t = work.tile([mn, D2 + 1], F32R, tag=f"ht1_{mc}", bufs=3)
                tf = work.tile([mn, D2], F32, tag="tf")
                nc.vector.tensor_reduce(tf[:],
                                        prod2[:].rearrange("p (o c) -> p o c", c=KC),
                                        axis=AX.X, op=OP.add)
                qd2 = work.tile([mn, D2], F32, tag="qd")
                nc.vector.tensor_tensor(qd2[:], tf[:], p2rep[:mn, :], op=OP.mult)
                with nc.allow_low_precision("fp32r feed of the msg2 matmul; "
                                            "sandbox-validated"):
                    nc.vector.tensor_copy(t[:, 0:D2], tf[:])
                    nc.vector.tensor_reduce(t[:, D2:D2 + 1], qd2[:], axis=AX.X,
                                            op=OP.add)
                ht2e.append(t)

            # msg2 (fp32r)
            msg2p = pacc.tile([D2 + 1, R], F32, tag="pacc")
            for jc, (jo, jn) in enumerate(CH):
                nc.tensor.matmul(msg2p[:], ht2e[jc][:], Qs[jc][:],
                                 start=(jc == 0), stop=(jc == 3))
            hT2m = work.tile([D2, R], F32, tag="hT2m")
            nc.vector.tensor_tensor(hT2m[:], msg2p[0:D2, :], f2[:], op=OP.mult)
            hT2 = work.tile([D2, R], F32, tag="hT1", bufs=3)
            nc.scalar.activation(hT2[:], hT2m[:], AF.Identity, bias=b2t[:])
            pre2r = work.tile([1, R], F32, tag="pre2r")
            nc.vector.tensor_tensor(pre2r[:], msg2p[D2:D2 + 1, :], f2[0:1, :],
                                    op=OP.mult)
            pre2f = work.tile([1, R], F32, tag="pre2f")
            nc.scalar.activation(pre2f[:], pre2r[:], AF.Identity, bias=pb2t[:])

            # pool2 (masked scores; flips here are benign)
            mp_row = work.tile([1, R], F32, tag="mp_row")
            nc.vector.scalar_tensor_tensor(mp_row[:], pre2f[:], SHIFT,
                                           keep_row[:], op0=OP.add, op1=OP.mult)
            mp_col = col_from_row(mp_row, "mp_col")
            keep2_col, _ = rank_keep(mp_row, mp_col, K2 - 0.5, "k2",
                                     want_row=False)
            s2_col = work.tile([128, 4], F32, tag="s_col")
            nc.scalar.activation(s2_col[:], mp_col[:], AF.Sigmoid,
                                 bias=nshift128[:])
            skk2_col = work.tile([128, 4], F32, tag="skk_col")
            nc.vector.tensor_tensor(skk2_col[:], s2_col[:], keep2_col[:],
                                    op=OP.mult)
            bB2_col = work.tile([128, 4], F32, tag="bB_col")
            nc.vector.tensor_scalar(bB2_col[:], keep2_col[:], BIG, -BIG,
                                    op0=OP.mult, op1=OP.add)
            readout(b, hT2, skk2_col, bB2_col, K2, R - K2, 2 * D1)

        seq = [bb for _ in range(reps) for bb in range(BL)]
        pending = None
        for b in seq:
            st = front(b)
            if pending is not None:
                back(*pending)
            pending = (b, st)
        back(*pending)

        from concourse.tile import add_dep_helper
        for k, tgt in (("g1", "pe"), ("red", "dv1"), ("red", "dv2")):
            if k in first_b:
                add_dep_helper(first_b[k].ins, fences[tgt].ins, sync=False,
                               reason="const fence ordering")

        # ---------------- AllGather + head (redundant on every core) --------
        zloc = dram.tile([128, BL], F32)
        zag = dram.tile([128 * n_cores, BL], F32)
        nc.gpsimd.dma_start(zloc[:], ztile[:])
        if n_cores == 1:
            nc.gpsimd.dma_start(zag[:], zloc[:])
        else:
            import concourse.mybir as _mb
            nc.gpsimd.collective_compute(
                "AllGather",
                _mb.AluOpType.bypass,
                replica_groups=[list(range(n_cores))],
                ins=[zloc[:].opt()],
                outs=[zag[:].opt()],
            )
        ZT = cons.tile([128, B], F32, tag="ZT")
        if n_cores == 1:
            nc.vector.memset(ZT[:], 0.0)
            nc.sync.dma_start(ZT[:, 0:BL], zag[:])
        else:
            nc.sync.dma_start(ZT[:].rearrange("p (c b) -> p c b", b=BL),
                              zag[:].rearrange("(c p) b -> p c b", p=128))

        def bn(y, n, gain, beta):
            mu = cons.tile([n, 1], F32, tag="bn_mu")
            nc.vector.tensor_reduce(mu[:], y[:], axis=AX.X, op=OP.add)
            nc.vector.tensor_scalar(mu[:], mu[:], 1.0 / B, None, op0=OP.mult)
            cen = cons.tile([n, B], F32, tag="bn_cen")
            nc.vector.tensor_scalar(cen[:], y[:], mu[:], None, op0=OP.subtract)
            sq = cons.tile([n, B], F32, tag="bn_sq")
            nc.vector.tensor_tensor(sq[:], cen[:], cen[:], op=OP.mult)
            var = cons.tile([n, 1], F32, tag="bn_var")
            nc.vector.tensor_reduce(var[:], sq[:], axis=AX.X, op=OP.add)
            rstd = cons.tile([n, 1], F32, tag="bn_rstd")
            nc.scalar.activation(rstd[:], var[:], AF.Sqrt, bias=eps128[:n, :],
                                 scale=1.0 / B)
            nc.vector.reciprocal(rstd[:], rstd[:])
            gn = cons.tile([n, 1], F32, tag="bn_gn")
            nc.vector.tensor_tensor(gn[:], rstd[:], gain, op=OP.mult)
            nc.vector.tensor_scalar(y[:], cen[:], gn[:], beta, op0=OP.mult, op1=OP.add)

        y1p = pg.tile([D2, B], F32, tag="pG")
        nc.tensor.matmul(y1p[:], fc1wt[:], ZT[:])
        y1 = cons.tile([D2, B], F32, tag="y1")
        nc.scalar.activation(y1[:], y1p[:], AF.Relu, bias=fc1bt[:])
        bn(y1, D2, g1t[:], be1t[:])

        y3p = pacc.tile([2, B], F32, tag="pacc")
        for mc in range(4):
            y2p = pg.tile([128, B], F32, tag="pG")
            nc.tensor.matmul(y2p[:], fc2wt[:, 128 * mc:128 * (mc + 1)], y1[:])
            y2 = cons.tile([128, B], F32, tag="y2")
            nc.scalar.activation(y2[:], y2p[:], AF.Relu, bias=fc2b4[:, mc:mc + 1])
            bn(y2, 128, g24[:, mc:mc + 1], be24[:, mc:mc + 1])
            nc.tensor.matmul(y3p[:], fc3wt[:, 2 * mc:2 * (mc + 1)], y2[:],
                             start=(mc == 0), stop=(mc == 3))
        y3 = cons.tile([2, B], F32, tag="y3")
        nc.scalar.activation(y3[:], y3p[:], AF.Identity, bias=fc3bt[:])
        nc.sync.dma_start(outd[:, :].rearrange("b o -> o b"), y3[:])

    # Walrus' MM descriptor holds a single sync wait; split multi-waits the
    # same way Bacc.compile does, then populate .instr bytes for extended
    # insts (reciprocal etc).
    import bass_rust as _br
    _br.move_matmul_waits_to_ldweights(nc.m)
    _br.generate_event_semaphores(nc)
    mybir.codegen_inst_isa_subclasses(nc)
    return nc


def make_in_maps(inputs, n_cores=NCORES):
    f32 = np.float32
    f16 = np.float16
    x = np.ascontiguousarray(inputs["x"], dtype=f32)
    adj = np.ascontiguousarray(inputs["adj_w"], dtype=f32)
    p1n = (inputs["p1"] / np.linalg.norm(inputs["p1"])).astype(f32)
    p2n = (inputs["p2"] / np.linalg.norm(inputs["p2"])).astype(f32)
    bc2 = np.ascontiguousarray(
        inputs["W2b"].reshape(KC, D1, D2).transpose(1, 2, 0).reshape(D1, D2 * KC), f32)
    bc2h = bc2.astype(f16)
    bc2l = (bc2 - bc2h.astype(f32)).astype(f16)
    shared = {
        "w1a": np.ascontiguousarray(inputs["W1a"], f32),
        "bc1": np.ascontiguousarray(
            inputs["W1b"].reshape(KC, R, D1).transpose(1, 2, 0).reshape(R, D1 * KC), f32),
        "b1d": np.ascontiguousarray(inputs["b1"], f32),
        "p1d": p1n,
        "pb1d": np.array([np.dot(p1n, inputs["b1"].astype(f32))], f32),
        "p1repd": np.ascontiguousarray(np.tile(p1n, (128, 1))),
        "p2repd": np.ascontiguousarray(np.tile(p2n, (128, 1))),
        "w2a": np.ascontiguousarray(inputs["W2a"], f32),
        "bc2hh": np.ascontiguousarray(np.concatenate([bc2h, bc2h], 0)),
        "bc2lo": np.ascontiguousarray(bc2l),
        "b2d": np.ascontiguousarray(inputs["b2"], f32),
        "p2d": p2n,
        "pb2d": np.array([np.dot(p2n, inputs["b2"].astype(f32))], f32),
        "fc1wd": np.ascontiguousarray(inputs["fc1_w"], f32),
        "fc1bd": np.ascontiguousarray(inputs["fc1_b"], f32),
        "g1d": np.ascontiguousarray(inputs["g1"], f32),
        "be1d": np.ascontiguousarray(inputs["be1"], f32),
        "fc2wd": np.ascontiguousarray(inputs["fc2_w"], f32),
        "fc2bd": np.ascontiguousarray(inputs["fc2_b"], f32),
        "g2d": np.ascontiguousarray(inputs["g2"], f32),
        "be2d": np.ascontiguousarray(inputs["be2"], f32),
        "fc3wd": np.ascontiguousarray(inputs["fc3_w"], f32),
        "fc3bd": np.ascontiguousarray(inputs["fc3_b"], f32),
    }
    cnt = 1.0 + np.asarray(inputs["adj_mask"], bool).sum(-1).astype(f32)
    rcl = (np.float32(1.0) / cnt).astype(f32)
    eye = np.eye(R, dtype=f32)
    BLc = B // n_cores
    maps = []
    for c in range(n_cores):
        m = dict(shared)
        sl = slice(c * BLc, (c + 1) * BLc)
        aI = adj[sl] + eye
        # (A+I)^T with columns j scaled by recip1[j]
        aIT = aI.transpose(0, 2, 1) * rcl[sl][:, None, :]
        m["xl"] = np.ascontiguousarray(x[sl])
        m["al"] = np.ascontiguousarray(aI)
        m["alT"] = np.ascontiguousarray(aIT.astype(f32))
        # recip1 in [128, 4] col-chunk layout
        r1p = np.zeros((BLc, 512), f32)
        r1p[:, :R] = rcl[sl]
        m["r1cl"] = np.ascontiguousarray(r1p.reshape(BLc, 4, 128).transpose(0, 2, 1))
        m["cnl"] = np.ascontiguousarray(cnt[sl])
        maps.append(m)
    return maps


_CACHED = {}


def _run_sim(in_maps):
    # Fallback executor: 8-core CoreSim of the same BIR.
    from concourse import bass_interp

    nc = build_nc(NCORES)
    sim = bass_interp.MultiCoreSim(nc, NCORES, num_workers=1)
    for i in range(NCORES):
        for k, v in in_maps[i].items():
            sim.cores[i].tensor(k)[:] = v
    sim.simulate()
    return np.array(sim.cores[0].tensor("out"), dtype=np.float32)


def kernel(**inputs):
    in_maps = make_in_maps(inputs, NCORES)
    try:
        from concourse.bass_utils import run_bass_kernel_spmd

        if "nc" not in _CACHED:
            _CACHED["nc"] = build_nc(NCORES)
        res = run_bass_kernel_spmd(_CACHED["nc"], in_maps, list(range(NCORES)))
        return np.asarray(res.results[0]["out"], dtype=np.float32)
    except Exception:
        return _run_sim(in_maps)
